# revision 1
# baseline (speedup 1.0000x reference)
"""BTV loss kernel for Trainium2 (8 NeuronCores, Bass/Tile).

reference: total = sum over 7x7 neighborhood shifts (k,l) != (0,0) of
           sqrt((x - roll(x,(k,l),axis=(2,3)))**2 + 1e-6).sum()
           out = 0.1 * total / x.size

Math used here:
  - circular-shift symmetry: shift (k,l) and (-k,-l) give identical sums,
    so only the 24 half-space shifts {k>0, any l} u {k==0, l>0} are
    computed and the result doubled.
  - sqrt(d^2 + 1e-6) ~= |d|: relative error of the final sum ~3e-6
    (verified numerically in f64), far below tolerance.
  - bf16 differences: |d| in bf16 adds ~1e-5 relative error (verified).

Pipeline per 128-row block (per core: 3 images x 8 blocks):
  - one DMA loads rows [128r, 128r+131) of all 3 images in bf16, twice
    (even + odd column phase) so every shifted view is 4B-aligned and
    DVE tensor ops run in 2x/4x packed modes.
  - DVE tensor_tensor subtract (bf16, 2x) per shift
  - |d| + free-dim reduce: split between ACT (activation Abs with
    accum_out, 1x but otherwise idle) and DVE (tensor_scalar abs_max 0
    with accum_out, 4x) to balance engine busy time.
  - per-partition partials accumulate in a (128, 192) f32 stage,
    reduced once at the end; host sums 8x128 values in f64.

Distribution: pure data parallel over the 24 (b,c) images, 3 per core.
"""

import dataclasses
import re
from operator import add as _py_add

import numpy as np

import concourse.bass as bass
import concourse.bacc as bacc_mod
import concourse.mybir as mybir
from concourse import dve_ops as _dvo
from concourse.dve_spec import AluOp as _DveAluOp
from concourse.dve_spec import Bin, Spec, Src0, Src1
from concourse.tile import TileContext
from concourse.bass_utils import run_bass_kernel_spmd

B, C, H, W = 8, 3, 1024, 1024
NCORES = 8
IMGS = (B * C) // NCORES        # images per core = 3
BASE = 4                        # left col pad (even => 4B-aligned in bf16)
WP = W + BASE + 3 + 1           # 1032: [w-4..w-1][0..1023][0,1,2][pad]
RB = 128                        # rows per block (partition dim)
NBLK = H // RB                  # 8 row blocks per image
ROWS_BLK = RB + 3               # 131 rows stored per block (128 + 3 halo)
# half-space shifts: (k>0, any l) or (k==0, l>0)
SHIFTS = [(k, l) for k in range(0, 4) for l in range(-3, 4) if (k > 0 or l > 0)]
assert len(SHIFTS) == 24
# which shifts run fully on DVE via the fused custom op ABS_DIFF_REDUCE
# (|a-b| + free-dim sum in one 1x instruction, ~3327ns) vs the split path
# (DVE bf16 2x subtract ~1669ns + ACT Abs/accum, batched 4 shifts per
# activation instruction to amortize the ~420ns fixed overhead).
FUSED = {2, 6, 10, 14, 18, 22}
ACT_GROUP = 4

WEIGHT = 0.1
F32 = mybir.dt.float32
BF16 = mybir.dt.bfloat16

_OPS_CACHE = None


def _mk_absdiff_uop(two_x: bool, kind: str):
    """One UopConfig for the absdiff-accumulate family.

    kind: "seed"   - first element(-pair) loads the accumulator (blk3)
          "steady" - accumulates into blk3's CURR_ALU_OUT flop
    1x datapath: blk0 |d|=ABSDIFF(lane0, chain0=SRC_1); blk1 captures |d|
    onto chain0, ALU bypasses; blk2 bypass; blk3 acc; blk4-7 bypass;
    WR0_LO <- DELAY_0.
    2x datapath: blk0 |d_lo|, blk1 |d_hi| (chains 1/2 = HI inputs),
    blk2 pair-sum (chain0=|d_lo|, captures |d_hi| on chain1); blk3 acc;
    WR0_LO <- DELAY_0, WR0_HI <- DELAY_1.
    """
    from concourse.dve_uop import (
        ENABLE,
        AluInp,
        DelayInp,
        InpSel,
        OutPath,
        OutSel,
        Trigger,
        UopConfig,
    )
    from concourse.dve_uop import AluOp as UAluOp

    seed = kind == "seed"
    u = UopConfig()
    u.enable_input(InpSel.SRC_0, 0)
    u.enable_input(InpSel.SRC_1, 1)
    if two_x:
        u.enable_input(InpSel.SRC_0_HI, 2)
        u.enable_input(InpSel.SRC_1_HI, 3)
    u.accum_enabled = ENABLE
    dp = u.datapath_config
    dp[0].enable_alu(UAluOp.ABSOLUTE_DIFF, AluInp.PREV_ALU_OUT, AluInp.PREV_DELAY_0)
    if two_x:
        dp[0].pass_through_delay(1, 2)
        dp[1].enable_alu(
            UAluOp.ABSOLUTE_DIFF, AluInp.PREV_DELAY_1, AluInp.PREV_DELAY_2
        )
        dp[1].enable_delay_from_src(DelayInp.PREV_ALU_OUT, 0)
        dp[2].enable_alu(UAluOp.ADD, AluInp.PREV_ALU_OUT, AluInp.PREV_DELAY_0)
        dp[2].enable_delay_from_src(DelayInp.PREV_ALU_OUT, 1)
        dp[2].pass_through_delay(0)
    else:
        dp[1].enable_alu(UAluOp.BYPASS, AluInp.PREV_ALU_OUT, AluInp.PREV_ALU_OUT)
        dp[1].enable_delay_from_src(DelayInp.PREV_ALU_OUT, 0)
        dp[2].enable_alu(UAluOp.BYPASS, AluInp.PREV_ALU_OUT, AluInp.PREV_ALU_OUT)
        dp[2].pass_through_delay(0)
    if seed:
        dp[3].enable_alu(UAluOp.BYPASS, AluInp.PREV_ALU_OUT, AluInp.PREV_ALU_OUT)
    else:
        dp[3].enable_alu(UAluOp.ADD, AluInp.CURR_ALU_OUT, AluInp.PREV_ALU_OUT)
    dp[3].pass_through_delay(0, 1) if two_x else dp[3].pass_through_delay(0)
    dp[3].alu_out_a_enable = ENABLE
    for b in (4, 5, 6, 7):
        dp[b].enable_alu(UAluOp.BYPASS, AluInp.PREV_ALU_OUT, AluInp.PREV_ALU_OUT)
        dp[b].pass_through_delay(0, 1) if two_x else dp[b].pass_through_delay(0)
        dp[b].alu_out_a_enable = ENABLE
    u.require_inp0 = ENABLE
    u.require_inp1 = ENABLE
    u.enable_output(OutSel.DELAY_0, OutPath.WR0_LO)
    if two_x:
        u.enable_output(OutSel.DELAY_1, OutPath.WR0_HI)
    if seed:
        u.trigger = (Trigger.COUNT, Trigger.SRC_TENSOR_DONE, Trigger.NONE)
        u.next_uop = (1, 0, 0)
        u.repeat_count = 1
    else:
        u.trigger = (Trigger.SRC_TENSOR_DONE, Trigger.NONE, Trigger.NONE)
        u.next_uop = (0, 0, 0)
    return u


def _mk_read_uop():
    """Single 1-element uop: route blk3's persistent accumulator flop to
    the output (blk3 BYPASS(CURR_ALU_OUT) -> blk4-7 bypass -> WR0_LO)."""
    from concourse.dve_uop import (
        ENABLE,
        AluInp,
        InpSel,
        OutPath,
        OutSel,
        Trigger,
        UopConfig,
    )
    from concourse.dve_uop import AluOp as UAluOp

    u = UopConfig()
    u.enable_input(InpSel.SRC_0, 0)
    dp = u.datapath_config
    for b in (0, 1, 2):
        dp[b].enable_alu(UAluOp.BYPASS, AluInp.PREV_ALU_OUT, AluInp.PREV_ALU_OUT)
    dp[3].enable_alu(UAluOp.BYPASS, AluInp.CURR_ALU_OUT, AluInp.CURR_ALU_OUT)
    for b in (4, 5, 6, 7):
        dp[b].enable_alu(UAluOp.BYPASS, AluInp.PREV_ALU_OUT, AluInp.PREV_ALU_OUT)
    u.require_inp0 = ENABLE
    u.enable_output(OutSel.ALU_OUT, OutPath.WR0_LO)
    u.trigger = (Trigger.SRC_TENSOR_DONE, Trigger.NONE, Trigger.NONE)
    u.next_uop = (0, 0, 0)
    return u


class _HandDveOp(_dvo.DveOp):
    """DveOp with hand-authored uop programs (1x and optional 2x)."""

    BUILDERS = {}  # name -> (build_1x, build_2x_or_None, rd1_en)

    def compile(self, ver):
        from concourse.dve_uop import DveOpSpec

        key = (self.name, ver)
        if (r := _dvo._COMPILE_CACHE.get(key)) is not None:
            return r
        b1, b2, rd1 = self.BUILDERS[self.name]
        result = DveOpSpec(
            name=self.name,
            opcode=_dvo.get_dve_sub_opcode(self.name),
            uops=b1(),
            uops_2x=(b2() if b2 is not None else None),
            rd1_en=rd1,
        )
        got = result.sha(ver)
        if self.uops_sha.get(ver) != got:
            raise ValueError(f"sha drift ({ver}: {got} != pinned)")
        _dvo._COMPILE_CACHE[key] = result
        return result


def _register(name, spec, build_1x, build_2x, rd1_en):
    _HandDveOp.BUILDERS[name] = (build_1x, build_2x, rd1_en)
    op = _HandDveOp(name, spec, subdim=False, uops_sha={})
    _dvo._SUB_OPCODE_FOR_NAME[name] = _dvo._CUSTOM_DVE_ROW_BASE + len(_dvo.OPS)
    shas = {}
    for ver in ("v3", "v4"):
        try:
            op.compile(ver)
            shas[ver] = op.uops_sha.get(ver)
        except ValueError as e:
            m = re.search(r"([0-9a-f]{16})", str(e))
            if not m:
                raise
            shas[ver] = m.group(1)
    op = dataclasses.replace(op, uops_sha=shas)
    _dvo.OPS.append(op)
    _dvo.CUSTOM_DVE_SPECS[name] = spec
    return op


def _get_ops():
    """Register (once per process) the three custom DVE ops and return
    (seed, cont, read)."""
    global _OPS_CACHE
    if _OPS_CACHE is not None:
        return _OPS_CACHE
    have = {op.name: op for op in _dvo.OPS}
    if "ABSDIFF_ACC_SEED_ANT" in have:
        _OPS_CACHE = (
            have["ABSDIFF_ACC_SEED_ANT"],
            have["ABSDIFF_ACC_CONT_ANT"],
            have["ABSDIFF_ACC_READ_ANT"],
        )
        return _OPS_CACHE

    def _ref_acc(in0, in1, s0, s1, imm2):
        b = np.abs(in0.astype(np.float32) - in1.astype(np.float32)).astype(
            np.float32
        )
        return b, b.reshape(b.shape[0], -1).sum(axis=-1, keepdims=True)

    spec_acc = Spec(
        body=Bin(_DveAluOp.ABSOLUTE_DIFF, Src0, Src1),
        accum=_py_add,
        reference=_ref_acc,
    )
    spec_read = Spec(
        body=Src0,
        reference=lambda in0, in1, s0, s1, imm2: in0.astype(np.float32),
    )
    seed = _register(
        "ABSDIFF_ACC_SEED_ANT",
        spec_acc,
        lambda: [_mk_absdiff_uop(False, "seed"), _mk_absdiff_uop(False, "steady")],
        lambda: [_mk_absdiff_uop(True, "seed"), _mk_absdiff_uop(True, "steady")],
        True,
    )
    cont = _register(
        "ABSDIFF_ACC_CONT_ANT",
        spec_acc,
        lambda: [_mk_absdiff_uop(False, "steady")],
        lambda: [_mk_absdiff_uop(True, "steady")],
        True,
    )
    read = _register(
        "ABSDIFF_ACC_READ_ANT",
        spec_read,
        lambda: [_mk_read_uop()],
        None,
        False,
    )
    _OPS_CACHE = (seed, cont, read)
    return _OPS_CACHE


CHAIN = 8  # fused ops per accumulator chain (one READ per chain)


def _build_nc():
    seed_op, cont_op, read_op = _get_ops()
    nc = bacc_mod.Bacc("TRN2", target_bir_lowering=False)
    # host layout: x[r, q, j, i, c] = pad_j[i, 128*r + q, c]; j=0 even
    # phase, j=1 odd phase (odd[c] = even[c+1]).
    X = nc.dram_tensor(
        "x", [NBLK, ROWS_BLK, IMGS, WP], BF16, kind="ExternalInput"
    )
    OUT = nc.dram_tensor("out", [128, 1], F32, kind="ExternalOutput")

    nsh = len(SHIFTS)
    nchain = (nsh + CHAIN - 1) // CHAIN
    row = IMGS * WP  # elements per stored row q
    with TileContext(nc) as tc:
        with (
            tc.tile_pool(name="ake", bufs=4) as ake_pool,
            tc.tile_pool(name="ako", bufs=3) as ako_pool,
            tc.tile_pool(name="sc", bufs=1) as sc_pool,
            tc.tile_pool(name="acc", bufs=1) as acc_pool,
        ):
            stage = acc_pool.tile([128, NBLK * nchain], F32)
            scratch = sc_pool.tile([128, IMGS, W], BF16)
            for r in range(NBLK):
                # ake[p,k,i,c] = X[r, p+k, i, c]; one DMA per block,
                # alternating between the two HWDGE rings.  The odd column
                # phase (ako[c] = ake[c+1], for bf16 4B alignment of odd-l
                # shifts) is derived on-device by a DVE 2x_2P copy instead
                # of shipping it over HBM.
                ake = ake_pool.tile([128, 4, IMGS, WP], BF16, tag="ake")
                ako = ako_pool.tile([128, 4, IMGS, WP], BF16, tag="ako")
                eng = nc.sync if r % 2 == 0 else nc.scalar
                eng.dma_start(
                    out=ake[:],
                    in_=bass.AP(
                        X,
                        r * ROWS_BLK * row,
                        [[row, 128], [1, 4 * row]],
                    ),
                )
                flat_e = ake[:].rearrange("p a b c -> p (a b c)")
                flat_o = ako[:].rearrange("p a b c -> p (a b c)")
                ncols = 4 * IMGS * WP
                nc.vector.tensor_copy(
                    out=flat_o[:, 0 : ncols - 2],
                    in_=flat_e[:, 1 : ncols - 1],
                )
                base = ake[:, 0, :, BASE : BASE + W]

                def shifted_ap(si):
                    k, l = SHIFTS[si]
                    if l % 2 == 0:
                        return ake[:, k, :, BASE + l : BASE + l + W]
                    return ako[:, k, :, BASE + l - 1 : BASE + l - 1 + W]

                for ci in range(nchain):
                    grp = list(range(ci * CHAIN, min((ci + 1) * CHAIN, nsh)))
                    col = r * nchain + ci
                    # the chain must run contiguously on DVE: the blk3
                    # accumulator flop carries state between instructions.
                    with tc.tile_critical():
                        for j, si in enumerate(grp):
                            bi = nc.vector._custom_dve(
                                seed_op if j == 0 else cont_op,
                                out=scratch[:],
                                in0=base,
                                in1=shifted_ap(si),
                            )
                            bi.ins.perf_max = 1
                        nc.vector._custom_dve(
                            read_op,
                            out=stage[:, col : col + 1],
                            in0=scratch[:, 0:1, 0:1],
                        )
            part = acc_pool.tile([128, 1], F32)
            nc.vector.tensor_reduce(
                out=part[:],
                in_=stage[:],
                axis=mybir.AxisListType.X,
                op=mybir.AluOpType.add,
            )
            nc.sync.dma_start(out=OUT[:], in_=part[:])
    return nc


_NC = None


def _get_nc():
    global _NC
    if _NC is None:
        _NC = _build_nc()
        if not _NC.is_finalized():
            _NC.finalize()
    return _NC


def _prep_shards(x: np.ndarray) -> list[dict[str, np.ndarray]]:
    """bf16-cast, circular pad, and blockify into the per-core
    (NBLK, 131, IMGS, WP) device layout (even phase only; the odd phase
    is derived on-device)."""
    imgs = np.ascontiguousarray(x.reshape(B * C, H, W), dtype=np.float32)

    def to_bf16(a32):
        b = a32.view(np.uint32)
        return ((b + 0x7FFF + ((b >> 16) & 1)) >> 16).astype(np.uint16)

    imgs_b = to_bf16(imgs)  # (24, H, W) uint16 view of bf16
    HPAD = H + 3
    even = np.zeros((B * C, HPAD, WP), dtype=np.uint16)
    even[:, :H, BASE : BASE + W] = imgs_b
    even[:, :H, :BASE] = imgs_b[:, :, W - BASE :]
    even[:, :H, BASE + W : BASE + W + 3] = imgs_b[:, :, :3]
    even[:, H:, :] = even[:, :3, :]

    shards = even.reshape(NCORES, IMGS, HPAD, WP)
    out = []
    for i in range(NCORES):
        t = shards[i].transpose(1, 0, 2)  # (HPAD, IMGS, WP)
        blk = np.empty((NBLK, ROWS_BLK, IMGS, WP), dtype=np.uint16)
        for r in range(NBLK):
            blk[r] = t[r * RB : r * RB + ROWS_BLK]
        out.append({"x": blk})
    return out


def _run(x: np.ndarray, trace: bool = False):
    import ml_dtypes

    nc = _get_nc()
    in_maps = _prep_shards(x)
    in_maps = [{"x": m["x"].view(ml_dtypes.bfloat16)} for m in in_maps]
    res = run_bass_kernel_spmd(
        nc, in_maps, core_ids=list(range(NCORES)), trace=trace
    )
    total = 0.0
    for r in res.results:
        total += r["out"].astype(np.float64).sum()
    val = WEIGHT * 2.0 * total / float(B * C * H * W)
    return np.float32(val), res


def kernel(x: np.ndarray) -> np.ndarray:
    x = np.asarray(x, dtype=np.float32)
    val, _ = _run(x, trace=False)
    return val



# revision 3
# speedup vs baseline: 1.1761x; 1.1761x over previous
"""BTV loss kernel for Trainium2 (8 NeuronCores, Bass/Tile).

reference: total = sum over 7x7 neighborhood shifts (k,l) != (0,0) of
           sqrt((x - roll(x,(k,l),axis=(2,3)))**2 + 1e-6).sum()
           out = 0.1 * total / x.size

Math:
  - circular-shift symmetry: shift (k,l) ~ (-k,-l); compute the 24
    half-space shifts {k>0, any l} u {k==0, l>0} and double.
  - sqrt(d^2 + 1e-6) ~= |d| (rel err ~3e-6); bf16 inputs add ~1e-5.

Engines (per 128-row block, 3 images per core):
  - DVE: custom CROSS op: one 2x instruction reads packed pairs of the
    base stream a and a shifted stream b (offset d) and accumulates
      |a_lo-b_lo| (shift d, even cols)   |a_hi-b_hi| (shift d, odd cols)
      |a_lo-b_hi| (shift d+1, even)      |a_hi-b_lo| (shift d-1, odd)
    i.e. 4 absdiff terms/cycle (2x the stock tensor_tensor rate). An
    11-instruction schedule covers shifts {k: l in -3..1} + (0,1..3)
    exactly (nd3/nd4 variants omit one cross term to avoid overlap).
  - PE+ACT: shifts (k,l) for k in 1..3, l in 2,3: PE computes
    diff = base - shift into PSUM via +I/-I matmuls; ACT does
    Abs + accum_out. 18 image-diffs/block in 9 pipelined groups of 2.
  - DMA: per block, tile_E (rows p, phase 0) and tile_O (rows p+j,
    j=0..3, columns shifted +1) are loaded straight from HBM; all
    odd column offsets come from tile_O so every DVE stream stays
    4-byte aligned with no on-chip repacking.

Distribution: pure data parallel over the 24 (b,c) images, 3 per core;
host sums the 8 per-core partials in f64.
"""

import dataclasses
import re
from operator import add as _py_add

import numpy as np

import concourse.bass as bass
import concourse.bacc as bacc_mod
import concourse.mybir as mybir
from concourse import dve_ops as _dvo
from concourse.dve_spec import AluOp as _DveAluOp
from concourse.dve_spec import Bin, Spec, Src0, Src1
from concourse.tile import TileContext
from concourse.bass_utils import run_bass_kernel_spmd

from concourse.dve_uop import (
    ENABLE,
    AluInp,
    AluOp as UAluOp,
    DelayInp,
    InpSel,
    OutPath,
    OutSel,
    Trigger,
    UopConfig,
)

B, C, H, W = 8, 3, 1024, 1024
NCORES = 8
IMGS = (B * C) // NCORES        # images per core = 3
BASE = 4                        # left col pad (even => 4B-aligned in bf16)
WP = W + BASE + 3 + 1           # 1032: [w-4..w-1][0..1023][0,1,2][pad]
RB = 128                        # rows per block (partition dim)
NBLK = H // RB                  # 8 row blocks per image
ROWS_BLK = RB + 3               # 131 rows stored per block (128 + 3 halo)
ROW = IMGS * WP                 # elements per stored row

WEIGHT = 0.1
F32 = mybir.dt.float32
BF16 = mybir.dt.bfloat16

# DVE schedule: (kind, variant, j=row phase, delta). Covers, per j>0,
# l in {-3..1} (ACT takes l=2,3), and for j=0 l in {1,2,3}:
#   full: F(d)+E(d+1)+O(d-1);  nd3: F(d)+O(d-1);  nd4: F(d)+E(d+1)
DVE_SCHED = [
    ("full", 0, 1),   # F(1), E(2), O(0)=0      [k=0]
    ("nd3", 0, 3),    # F(3), O(2)              [k=0]
]
for _j in (1, 2, 3):
    DVE_SCHED += [
        ("nd4", _j, -3),   # F(-3), E(-2)
        ("full", _j, -1),  # F(-1), E(0), O(-2)
        ("nd3", _j, 1),    # F(1), O(0)
    ]
# PE/ACT shifts: (j, l) pairs
ACT_SHIFTS = [(j, l) for j in (1, 2, 3) for l in (2, 3)]
ACT_GRP = 2  # image-diffs per PSUM tile / ACT instruction


def _mk_cross_uop(kind: str, use_d3: bool, use_d4: bool):
    """2x CROSS uop. kind: "seed" (acc <- sum on elem 0) | "steady".
    blocks: 0:d1  1:d4  2:d3  3:d2  4..6:sum tree  7:acc
    Omitted terms compute ABSDIFF(x, x) = 0 instead (same structure).
    """
    u = UopConfig()
    u.enable_input(InpSel.SRC_0, 0)      # a_lo -> ALU lane
    u.enable_input(InpSel.SRC_1, 1)      # b_lo -> delay lane 0
    u.enable_input(InpSel.SRC_0_HI, 2)   # a_hi -> delay lane 1
    u.enable_input(InpSel.SRC_1_HI, 3)   # b_hi -> delay lane 2
    u.accum_enabled = ENABLE
    dp = u.datapath_config
    dp[0].enable_alu(UAluOp.ABSOLUTE_DIFF, AluInp.PREV_ALU_OUT, AluInp.PREV_DELAY_0)
    dp[0].enable_delay_from_src(DelayInp.PREV_ALU_OUT, 3)
    dp[0].pass_through_delay(0, 1, 2)
    dp[1].enable_alu(
        UAluOp.ABSOLUTE_DIFF,
        AluInp.PREV_DELAY_1,
        AluInp.PREV_DELAY_0 if use_d4 else AluInp.PREV_DELAY_1,
    )
    dp[1].enable_delay_from_src(DelayInp.PREV_ALU_OUT, 0)
    dp[1].pass_through_delay(1, 2, 3)
    dp[2].enable_alu(
        UAluOp.ABSOLUTE_DIFF,
        AluInp.PREV_DELAY_3,
        AluInp.PREV_DELAY_2 if use_d3 else AluInp.PREV_DELAY_3,
    )
    dp[2].enable_delay_from_src(DelayInp.PREV_ALU_OUT, 3)
    dp[2].pass_through_delay(0, 1, 2)
    dp[3].enable_alu(UAluOp.ABSOLUTE_DIFF, AluInp.PREV_DELAY_1, AluInp.PREV_DELAY_2)
    dp[3].enable_delay_from_src(DelayInp.PREV_ALU_OUT, 1)
    dp[3].pass_through_delay(0, 3)
    dp[4].enable_alu(UAluOp.ADD, AluInp.PREV_ALU_OUT, AluInp.PREV_DELAY_1)
    dp[4].pass_through_delay(0, 3)
    dp[5].enable_alu(UAluOp.ADD, AluInp.PREV_ALU_OUT, AluInp.PREV_DELAY_0)
    dp[5].pass_through_delay(3)
    dp[6].enable_alu(UAluOp.ADD, AluInp.PREV_ALU_OUT, AluInp.PREV_DELAY_3)
    if kind == "seed":
        dp[7].enable_alu(UAluOp.BYPASS, AluInp.PREV_ALU_OUT, AluInp.PREV_ALU_OUT)
    else:
        dp[7].enable_alu(UAluOp.ADD, AluInp.CURR_ALU_OUT, AluInp.PREV_ALU_OUT)
    dp[7].alu_out_a_enable = ENABLE
    u.require_inp0 = ENABLE
    u.require_inp1 = ENABLE
    u.enable_output(OutSel.ALU_OUT, OutPath.WR0_LO)
    u.enable_output(OutSel.ALU_OUT, OutPath.WR0_HI)
    if kind == "seed":
        u.trigger = (Trigger.COUNT, Trigger.SRC_TENSOR_DONE, Trigger.NONE)
        u.next_uop = (1, 0, 0)
        u.repeat_count = 1
    else:
        u.trigger = (Trigger.SRC_TENSOR_DONE, Trigger.NONE, Trigger.NONE)
        u.next_uop = (0, 0, 0)
    return u


def _mk_poison_1x():
    """1x fallback: acc <- +inf so any non-2x execution is caught."""
    u = UopConfig()
    u.enable_input(InpSel.SRC_0, 0)
    u.enable_input(InpSel.POS_INF, 1)
    dp = u.datapath_config
    for b in range(7):
        dp[b].enable_alu(UAluOp.BYPASS, AluInp.PREV_ALU_OUT, AluInp.PREV_ALU_OUT)
        dp[b].pass_through_delay(0)
    dp[7].enable_alu(UAluOp.BYPASS, AluInp.PREV_DELAY_0, AluInp.PREV_DELAY_0)
    dp[7].alu_out_a_enable = ENABLE
    u.accum_enabled = ENABLE
    u.require_inp0 = ENABLE
    u.require_inp1 = ENABLE
    u.enable_output(OutSel.ALU_OUT, OutPath.WR0_LO)
    u.trigger = (Trigger.SRC_TENSOR_DONE, Trigger.NONE, Trigger.NONE)
    u.next_uop = (0, 0, 0)
    return u


def _mk_read_uop():
    """Route blk7's accumulator flop to the output (1-element stream)."""
    u = UopConfig()
    u.enable_input(InpSel.SRC_0, 0)
    dp = u.datapath_config
    for b in range(7):
        dp[b].enable_alu(UAluOp.BYPASS, AluInp.PREV_ALU_OUT, AluInp.PREV_ALU_OUT)
    dp[7].enable_alu(UAluOp.BYPASS, AluInp.CURR_ALU_OUT, AluInp.CURR_ALU_OUT)
    u.require_inp0 = ENABLE
    u.enable_output(OutSel.ALU_OUT, OutPath.WR0_LO)
    u.trigger = (Trigger.SRC_TENSOR_DONE, Trigger.NONE, Trigger.NONE)
    u.next_uop = (0, 0, 0)
    return u


class _HandDveOp(_dvo.DveOp):
    BUILDERS = {}  # name -> (build_1x_list, build_2x_list_or_None, rd1_en)

    def compile(self, ver):
        from concourse.dve_uop import DveOpSpec

        key = (self.name, ver)
        if (r := _dvo._COMPILE_CACHE.get(key)) is not None:
            return r
        b1, b2, rd1 = self.BUILDERS[self.name]
        result = DveOpSpec(
            name=self.name,
            opcode=_dvo.get_dve_sub_opcode(self.name),
            uops=b1(),
            uops_2x=(b2() if b2 is not None else None),
            rd1_en=rd1,
        )
        got = result.sha(ver)
        if self.uops_sha.get(ver) != got:
            raise ValueError(f"sha drift ({ver}: {got} != pinned)")
        _dvo._COMPILE_CACHE[key] = result
        return result


def _register(name, spec, build_1x, build_2x, rd1_en):
    _HandDveOp.BUILDERS[name] = (build_1x, build_2x, rd1_en)
    op = _HandDveOp(name, spec, subdim=False, uops_sha={})
    _dvo._SUB_OPCODE_FOR_NAME[name] = _dvo._CUSTOM_DVE_ROW_BASE + len(_dvo.OPS)
    shas = {}
    for ver in ("v3", "v4"):
        try:
            op.compile(ver)
            shas[ver] = op.uops_sha.get(ver)
        except ValueError as e:
            m = re.search(r"([0-9a-f]{16})", str(e))
            if not m:
                raise
            shas[ver] = m.group(1)
    op = dataclasses.replace(op, uops_sha=shas)
    _dvo.OPS.append(op)
    _dvo.CUSTOM_DVE_SPECS[name] = spec
    return op


_OPS = None


def _get_ops():
    """dict: (kind, variant) -> op, plus 'read'."""
    global _OPS
    if _OPS is not None:
        return _OPS
    have = {op.name: op for op in _dvo.OPS}
    names = {
        ("seed", "full"): "XR_SEED_F_ANT",
        ("seed", "nd3"): "XR_SEED_ND3_ANT",
        ("seed", "nd4"): "XR_SEED_ND4_ANT",
        ("cont", "full"): "XR_CONT_F_ANT",
        ("cont", "nd3"): "XR_CONT_ND3_ANT",
        ("cont", "nd4"): "XR_CONT_ND4_ANT",
    }
    if names[("seed", "full")] in have:
        _OPS = {k: have[n] for k, n in names.items()}
        _OPS["read"] = have["XR_READ_ANT"]
        return _OPS

    def _ref(in0, in1, s0, s1, imm2):
        a = in0.astype(np.float32)
        b = in1.astype(np.float32)
        P = a.shape[0]
        out = np.abs(a.reshape(P, -1) - b.reshape(P, -1))
        return out.reshape(in0.shape), out.reshape(P, -1).sum(-1, keepdims=True)

    spec_acc = Spec(
        body=Bin(_DveAluOp.ABSOLUTE_DIFF, Src0, Src1),
        accum=_py_add,
        reference=_ref,
    )
    spec_read = Spec(
        body=Src0,
        reference=lambda in0, in1, s0, s1, imm2: in0.astype(np.float32),
    )
    _OPS = {}
    for (kind, var), name in names.items():
        d3, d4 = var != "nd3", var != "nd4"
        _OPS[(kind, var)] = _register(
            name,
            spec_acc,
            lambda: [_mk_poison_1x(), _mk_poison_1x()],
            lambda kind=kind, d3=d3, d4=d4: [
                _mk_cross_uop(kind, d3, d4),
                _mk_cross_uop("steady", d3, d4),
            ],
            True,
        )
    _OPS["read"] = _register(
        "XR_READ_ANT", spec_read, lambda: [_mk_read_uop()], None, False
    )
    return _OPS


NGRP = (len(ACT_SHIFTS) * IMGS) // ACT_GRP  # ACT groups per block = 9
STAGE_COLS = NBLK * (1 + NGRP)              # 80


def _build_nc():
    ops = _get_ops()
    nc = bacc_mod.Bacc("TRN2", target_bir_lowering=False)
    # host layout: flat; stored[r, q, i, c] = block r, row 128r+q, img i,
    # col c (with BASE left pad / 4 right cols); +8 elements tail pad so
    # the tile_O DMA (+1 element offset) stays in bounds.
    X = nc.dram_tensor(
        "x", [NBLK * ROWS_BLK * ROW + 8], BF16, kind="ExternalInput"
    )
    WI = nc.dram_tensor("wi", [128, 128], BF16, kind="ExternalInput")
    WNI = nc.dram_tensor("wni", [128, 128], BF16, kind="ExternalInput")
    OUT = nc.dram_tensor("out", [128, 1], F32, kind="ExternalOutput")

    with TileContext(nc) as tc:
        with (
            tc.tile_pool(name="te", bufs=3) as te_pool,
            tc.tile_pool(name="to", bufs=3) as to_pool,
            tc.tile_pool(name="sc", bufs=1) as sc_pool,
            tc.tile_pool(name="acc", bufs=1) as acc_pool,
            tc.psum_pool(name="ps", bufs=2) as ps_pool,
        ):
            stage = acc_pool.tile([128, STAGE_COLS], F32)
            scratch = sc_pool.tile([128, IMGS, W], BF16)
            ascr = acc_pool.tile([128, ACT_GRP * W], BF16)
            wi = acc_pool.tile([128, 128], BF16)
            wni = acc_pool.tile([128, 128], BF16)
            nc.sync.dma_start(out=wi[:], in_=WI[:])
            nc.sync.dma_start(out=wni[:], in_=WNI[:])
            # pre-load the ACT Abs table before any DVE critical section
            # (the lazy table-load DMA deadlocks against critical branches)
            nc.scalar.activation(
                out=ascr[:, 0:2],
                in_=wi[:, 0:2],
                func=mybir.ActivationFunctionType.Abs,
            )
            for r in range(NBLK):
                te = te_pool.tile([128, IMGS, WP], BF16, tag="te")
                to = to_pool.tile([128, 4, IMGS, WP], BF16, tag="to")
                eng = nc.sync if r % 2 == 0 else nc.scalar
                eng.dma_start(
                    out=te[:],
                    in_=bass.AP(X, r * ROWS_BLK * ROW, [[ROW, 128], [1, ROW]]),
                )
                eng.dma_start(
                    out=to[:],
                    in_=bass.AP(
                        X, r * ROWS_BLK * ROW + 1, [[ROW, 128], [1, 4 * ROW]]
                    ),
                )
                base = te[:, :, BASE : BASE + W]

                def bview(j, delta):
                    c0 = BASE + delta - 1
                    return to[:, j, :, c0 : c0 + W]

                # --- PE + ACT: shifts (j, l) for l in {2, 3} ---
                diffs = [(j, l, i) for (j, l) in ACT_SHIFTS for i in range(IMGS)]
                MMW = 512  # matmul free-dim cap (one PSUM bank)
                for g in range(NGRP):
                    psum = ps_pool.tile([128, ACT_GRP * W], F32, tag="ps")
                    # all +I passes first, then all -I (amortize weight loads)
                    for m in range(ACT_GRP):
                        j, l, i = diffs[g * ACT_GRP + m]
                        for c0 in range(0, W, MMW):
                            nc.tensor.matmul(
                                out=psum[:, m * W + c0 : m * W + c0 + MMW],
                                lhsT=wi[:],
                                rhs=te[:, i, BASE + c0 : BASE + c0 + MMW],
                                start=True,
                                stop=False,
                            )
                    for m in range(ACT_GRP):
                        j, l, i = diffs[g * ACT_GRP + m]
                        b0 = BASE + l - 1
                        for c0 in range(0, W, MMW):
                            nc.tensor.matmul(
                                out=psum[:, m * W + c0 : m * W + c0 + MMW],
                                lhsT=wni[:],
                                rhs=to[:, j, i, b0 + c0 : b0 + c0 + MMW],
                                start=False,
                                stop=True,
                            )
                    col = r * (1 + NGRP) + 1 + g
                    nc.scalar.activation(
                        out=ascr[:],
                        in_=psum[:],
                        func=mybir.ActivationFunctionType.Abs,
                        accum_out=stage[:, col : col + 1],
                    )

                # --- DVE chain: 11 cross instructions + read ---
                with tc.tile_critical():
                    for n, (var, j, delta) in enumerate(DVE_SCHED):
                        kind = "seed" if n == 0 else "cont"
                        nc.vector._custom_dve(
                            ops[(kind, var)],
                            out=scratch[:],
                            in0=base,
                            in1=bview(j, delta),
                        ).ins.perf_max = 1
                    col = r * (1 + NGRP)
                    nc.vector._custom_dve(
                        ops["read"],
                        out=stage[:, col : col + 1],
                        in0=scratch[:, 0:1, 0:1],
                    )
            part = acc_pool.tile([128, 1], F32)
            nc.vector.tensor_reduce(
                out=part[:],
                in_=stage[:],
                axis=mybir.AxisListType.X,
                op=mybir.AluOpType.add,
            )
            nc.sync.dma_start(out=OUT[:], in_=part[:])
    return nc


_NC = None


def _get_nc():
    global _NC
    if _NC is None:
        _NC = _build_nc()
        if not _NC.is_finalized():
            _NC.finalize()
    return _NC


def _prep_shards(x: np.ndarray) -> list[dict[str, np.ndarray]]:
    """bf16-cast, circular pad, blockify into the flat per-core layout."""
    imgs = np.ascontiguousarray(x.reshape(B * C, H, W), dtype=np.float32)

    def to_bf16(a32):
        b = a32.view(np.uint32)
        return ((b + 0x7FFF + ((b >> 16) & 1)) >> 16).astype(np.uint16)

    imgs_b = to_bf16(imgs)  # (24, H, W) uint16 view of bf16
    HPAD = H + 3
    even = np.zeros((B * C, HPAD, WP), dtype=np.uint16)
    even[:, :H, BASE : BASE + W] = imgs_b
    even[:, :H, :BASE] = imgs_b[:, :, W - BASE :]
    even[:, :H, BASE + W : BASE + W + 3] = imgs_b[:, :, :3]
    even[:, H:, :] = even[:, :3, :]

    I = np.eye(128, dtype=np.float32)
    wi = to_bf16(I)
    wni = to_bf16(-I)

    shards = even.reshape(NCORES, IMGS, HPAD, WP)
    out = []
    for n in range(NCORES):
        t = shards[n].transpose(1, 0, 2)  # (HPAD, IMGS, WP)
        blk = np.empty((NBLK, ROWS_BLK, IMGS, WP), dtype=np.uint16)
        for r in range(NBLK):
            blk[r] = t[r * RB : r * RB + ROWS_BLK]
        flat = np.concatenate([blk.reshape(-1), np.zeros(8, np.uint16)])
        out.append({"x": flat, "wi": wi, "wni": wni})
    return out


def _run(x: np.ndarray, trace: bool = False):
    import ml_dtypes

    nc = _get_nc()
    in_maps = _prep_shards(x)
    in_maps = [
        {k: v.view(ml_dtypes.bfloat16) for k, v in m.items()} for m in in_maps
    ]
    res = run_bass_kernel_spmd(
        nc, in_maps, core_ids=list(range(NCORES)), trace=trace
    )
    total = 0.0
    for r in res.results:
        total += r["out"].astype(np.float64).sum()
    val = WEIGHT * 2.0 * total / float(B * C * H * W)
    return np.float32(val), res


def kernel(x: np.ndarray) -> np.ndarray:
    x = np.asarray(x, dtype=np.float32)
    val, _ = _run(x, trace=False)
    return val


# revision 4
# speedup vs baseline: 2.2121x; 1.8808x over previous
"""BTV loss kernel for Trainium2 (8 NeuronCores, Bass/Tile).

reference: total = sum over 7x7 neighborhood shifts (k,l) != (0,0) of
           sqrt((x - roll(x,(k,l),axis=(2,3)))**2 + 1e-6).sum()
           out = 0.1 * total / x.size

Math:
  - circular-shift symmetry: shift (k,l) ~ (-k,-l); compute the 24
    half-space shifts {k>0, any l} u {k==0, l>0} and double.
  - sqrt(d^2 + 1e-6) ~= |d| (rel err ~3e-6); bf16 inputs add ~1e-5.

Engines (per 128-row block, 3 images per core):
  - DVE: custom CROSS op: one 2x instruction reads packed pairs of the
    base stream a and a shifted stream b (offset d) and accumulates
      |a_lo-b_lo| (shift d, even cols)   |a_hi-b_hi| (shift d, odd cols)
      |a_lo-b_hi| (shift d+1, even)      |a_hi-b_lo| (shift d-1, odd)
    i.e. 4 absdiff terms/cycle (2x the stock tensor_tensor rate). An
    11-instruction schedule covers shifts {k: l in -3..1} + (0,1..3)
    exactly (nd3/nd4 variants omit one cross term to avoid overlap).
  - PE+ACT: shifts (k,l) for k in 1..3, l in 2,3: PE computes
    diff = base - shift into PSUM via +I/-I matmuls; ACT does
    Abs + accum_out. 18 image-diffs/block in 9 pipelined groups of 2.
  - DMA: per block, tile_E (rows p, phase 0) and tile_O (rows p+j,
    j=0..3, columns shifted +1) are loaded straight from HBM; all
    odd column offsets come from tile_O so every DVE stream stays
    4-byte aligned with no on-chip repacking.

Distribution: pure data parallel over the 24 (b,c) images, 3 per core;
host sums the 8 per-core partials in f64.
"""

import dataclasses
import re
from operator import add as _py_add

import numpy as np

import concourse.bass as bass
import concourse.bacc as bacc_mod
import concourse.mybir as mybir
from concourse import dve_ops as _dvo
from concourse.dve_spec import AluOp as _DveAluOp
from concourse.dve_spec import Bin, Spec, Src0, Src1
from concourse.tile import TileContext
from concourse.bass_utils import run_bass_kernel_spmd

from concourse.dve_uop import (
    ENABLE,
    AluInp,
    AluOp as UAluOp,
    DelayInp,
    InpSel,
    OutPath,
    OutSel,
    Trigger,
    UopConfig,
)

B, C, H, W = 8, 3, 1024, 1024
NCORES = 8
IMGS = (B * C) // NCORES        # images per core = 3
BASE = 4                        # left col pad (even => 4B-aligned in bf16)
WP = W + BASE + 3 + 1           # 1032: [w-4..w-1][0..1023][0,1,2][pad]
RB = 128                        # rows per block (partition dim)
NBLK = H // RB                  # 8 row blocks per image
ROWS_BLK = RB + 3               # 131 rows stored per block (128 + 3 halo)
ROW = IMGS * WP                 # elements per stored row

WEIGHT = 0.1
F32 = mybir.dt.float32
BF16 = mybir.dt.bfloat16

# DVE schedule: (kind, variant, j=row phase, delta). Covers, per j>0,
# l in {-3..1} (ACT takes l=2,3), and for j=0 l in {1,2,3}:
#   full: F(d)+E(d+1)+O(d-1);  nd3: F(d)+O(d-1);  nd4: F(d)+E(d+1)
DVE_SCHED = [
    ("full", 0, 1),   # F(1), E(2), O(0)=0      [k=0]
    ("nd3", 0, 3),    # F(3), O(2)              [k=0]
]
for _j in (1, 2, 3):
    DVE_SCHED += [
        ("nd4", _j, -3),   # F(-3), E(-2)
        ("full", _j, -1),  # F(-1), E(0), O(-2)
        ("nd3", _j, 1),    # F(1), O(0)
    ]
# PE/ACT shifts: (j, l) pairs
ACT_SHIFTS = [(j, l) for j in (1, 2, 3) for l in (2, 3)]
ACT_GRP = 2  # image-diffs per PSUM tile / ACT instruction


def _mk_cross_uop(kind: str, use_d3: bool, use_d4: bool):
    """2x CROSS uop. kind: "seed" (acc <- sum on elem 0) | "steady".
    blocks: 0:d1  1:d4  2:d3  3:d2  4..6:sum tree  7:acc
    Omitted terms compute ABSDIFF(x, x) = 0 instead (same structure).
    """
    u = UopConfig()
    u.enable_input(InpSel.SRC_0, 0)      # a_lo -> ALU lane
    u.enable_input(InpSel.SRC_1, 1)      # b_lo -> delay lane 0
    u.enable_input(InpSel.SRC_0_HI, 2)   # a_hi -> delay lane 1
    u.enable_input(InpSel.SRC_1_HI, 3)   # b_hi -> delay lane 2
    u.accum_enabled = ENABLE
    dp = u.datapath_config
    dp[0].enable_alu(UAluOp.ABSOLUTE_DIFF, AluInp.PREV_ALU_OUT, AluInp.PREV_DELAY_0)
    dp[0].enable_delay_from_src(DelayInp.PREV_ALU_OUT, 3)
    dp[0].pass_through_delay(0, 1, 2)
    dp[1].enable_alu(
        UAluOp.ABSOLUTE_DIFF,
        AluInp.PREV_DELAY_1,
        AluInp.PREV_DELAY_0 if use_d4 else AluInp.PREV_DELAY_1,
    )
    dp[1].enable_delay_from_src(DelayInp.PREV_ALU_OUT, 0)
    dp[1].pass_through_delay(1, 2, 3)
    dp[2].enable_alu(
        UAluOp.ABSOLUTE_DIFF,
        AluInp.PREV_DELAY_3,
        AluInp.PREV_DELAY_2 if use_d3 else AluInp.PREV_DELAY_3,
    )
    dp[2].enable_delay_from_src(DelayInp.PREV_ALU_OUT, 3)
    dp[2].pass_through_delay(0, 1, 2)
    dp[3].enable_alu(UAluOp.ABSOLUTE_DIFF, AluInp.PREV_DELAY_1, AluInp.PREV_DELAY_2)
    dp[3].enable_delay_from_src(DelayInp.PREV_ALU_OUT, 1)
    dp[3].pass_through_delay(0, 3)
    dp[4].enable_alu(UAluOp.ADD, AluInp.PREV_ALU_OUT, AluInp.PREV_DELAY_1)
    dp[4].pass_through_delay(0, 3)
    dp[5].enable_alu(UAluOp.ADD, AluInp.PREV_ALU_OUT, AluInp.PREV_DELAY_0)
    dp[5].pass_through_delay(3)
    dp[6].enable_alu(UAluOp.ADD, AluInp.PREV_ALU_OUT, AluInp.PREV_DELAY_3)
    if kind == "seed":
        dp[7].enable_alu(UAluOp.BYPASS, AluInp.PREV_ALU_OUT, AluInp.PREV_ALU_OUT)
    else:
        dp[7].enable_alu(UAluOp.ADD, AluInp.CURR_ALU_OUT, AluInp.PREV_ALU_OUT)
    dp[7].alu_out_a_enable = ENABLE
    u.require_inp0 = ENABLE
    u.require_inp1 = ENABLE
    u.enable_output(OutSel.ALU_OUT, OutPath.WR0_LO)
    u.enable_output(OutSel.ALU_OUT, OutPath.WR0_HI)
    if kind == "seed":
        u.trigger = (Trigger.COUNT, Trigger.SRC_TENSOR_DONE, Trigger.NONE)
        u.next_uop = (1, 0, 0)
        u.repeat_count = 1
    else:
        u.trigger = (Trigger.SRC_TENSOR_DONE, Trigger.NONE, Trigger.NONE)
        u.next_uop = (0, 0, 0)
    return u


def _mk_poison_1x():
    """1x fallback: acc <- +inf so any non-2x execution is caught."""
    u = UopConfig()
    u.enable_input(InpSel.SRC_0, 0)
    u.enable_input(InpSel.POS_INF, 1)
    dp = u.datapath_config
    for b in range(7):
        dp[b].enable_alu(UAluOp.BYPASS, AluInp.PREV_ALU_OUT, AluInp.PREV_ALU_OUT)
        dp[b].pass_through_delay(0)
    dp[7].enable_alu(UAluOp.BYPASS, AluInp.PREV_DELAY_0, AluInp.PREV_DELAY_0)
    dp[7].alu_out_a_enable = ENABLE
    u.accum_enabled = ENABLE
    u.require_inp0 = ENABLE
    u.require_inp1 = ENABLE
    u.enable_output(OutSel.ALU_OUT, OutPath.WR0_LO)
    u.trigger = (Trigger.SRC_TENSOR_DONE, Trigger.NONE, Trigger.NONE)
    u.next_uop = (0, 0, 0)
    return u


def _mk_read_uop():
    """Route blk7's accumulator flop to the output (1-element stream)."""
    u = UopConfig()
    u.enable_input(InpSel.SRC_0, 0)
    dp = u.datapath_config
    for b in range(7):
        dp[b].enable_alu(UAluOp.BYPASS, AluInp.PREV_ALU_OUT, AluInp.PREV_ALU_OUT)
    dp[7].enable_alu(UAluOp.BYPASS, AluInp.CURR_ALU_OUT, AluInp.CURR_ALU_OUT)
    u.require_inp0 = ENABLE
    u.enable_output(OutSel.ALU_OUT, OutPath.WR0_LO)
    u.trigger = (Trigger.SRC_TENSOR_DONE, Trigger.NONE, Trigger.NONE)
    u.next_uop = (0, 0, 0)
    return u


class _HandDveOp(_dvo.DveOp):
    BUILDERS = {}  # name -> (build_1x_list, build_2x_list_or_None, rd1_en)

    def compile(self, ver):
        from concourse.dve_uop import DveOpSpec

        key = (self.name, ver)
        if (r := _dvo._COMPILE_CACHE.get(key)) is not None:
            return r
        b1, b2, rd1 = self.BUILDERS[self.name]
        result = DveOpSpec(
            name=self.name,
            opcode=_dvo.get_dve_sub_opcode(self.name),
            uops=b1(),
            uops_2x=(b2() if b2 is not None else None),
            rd1_en=rd1,
        )
        got = result.sha(ver)
        if self.uops_sha.get(ver) != got:
            raise ValueError(f"sha drift ({ver}: {got} != pinned)")
        _dvo._COMPILE_CACHE[key] = result
        return result


def _register(name, spec, build_1x, build_2x, rd1_en):
    _HandDveOp.BUILDERS[name] = (build_1x, build_2x, rd1_en)
    op = _HandDveOp(name, spec, subdim=False, uops_sha={})
    _dvo._SUB_OPCODE_FOR_NAME[name] = _dvo._CUSTOM_DVE_ROW_BASE + len(_dvo.OPS)
    shas = {}
    for ver in ("v3", "v4"):
        try:
            op.compile(ver)
            shas[ver] = op.uops_sha.get(ver)
        except ValueError as e:
            m = re.search(r"([0-9a-f]{16})", str(e))
            if not m:
                raise
            shas[ver] = m.group(1)
    op = dataclasses.replace(op, uops_sha=shas)
    _dvo.OPS.append(op)
    _dvo.CUSTOM_DVE_SPECS[name] = spec
    return op


_OPS = None


def _get_ops():
    """dict: (kind, variant) -> op, plus 'read'."""
    global _OPS
    if _OPS is not None:
        return _OPS
    have = {op.name: op for op in _dvo.OPS}
    names = {
        ("seed", "full"): "XR_SEED_F_ANT",
        ("seed", "nd3"): "XR_SEED_ND3_ANT",
        ("seed", "nd4"): "XR_SEED_ND4_ANT",
        ("cont", "full"): "XR_CONT_F_ANT",
        ("cont", "nd3"): "XR_CONT_ND3_ANT",
        ("cont", "nd4"): "XR_CONT_ND4_ANT",
    }
    if names[("seed", "full")] in have:
        _OPS = {k: have[n] for k, n in names.items()}
        _OPS["read"] = have["XR_READ_ANT"]
        return _OPS

    def _ref(in0, in1, s0, s1, imm2):
        a = in0.astype(np.float32)
        b = in1.astype(np.float32)
        P = a.shape[0]
        out = np.abs(a.reshape(P, -1) - b.reshape(P, -1))
        return out.reshape(in0.shape), out.reshape(P, -1).sum(-1, keepdims=True)

    spec_acc = Spec(
        body=Bin(_DveAluOp.ABSOLUTE_DIFF, Src0, Src1),
        accum=_py_add,
        reference=_ref,
    )
    spec_read = Spec(
        body=Src0,
        reference=lambda in0, in1, s0, s1, imm2: in0.astype(np.float32),
    )
    _OPS = {}
    for (kind, var), name in names.items():
        d3, d4 = var != "nd3", var != "nd4"
        _OPS[(kind, var)] = _register(
            name,
            spec_acc,
            lambda: [_mk_poison_1x(), _mk_poison_1x()],
            lambda kind=kind, d3=d3, d4=d4: [
                _mk_cross_uop(kind, d3, d4),
                _mk_cross_uop("steady", d3, d4),
            ],
            True,
        )
    _OPS["read"] = _register(
        "XR_READ_ANT", spec_read, lambda: [_mk_read_uop()], None, False
    )
    return _OPS


NGRP = (len(ACT_SHIFTS) * IMGS) // ACT_GRP  # ACT groups per block = 9
STAGE_COLS = NBLK * (1 + NGRP)              # 80


def _build_nc():
    ops = _get_ops()
    nc = bacc_mod.Bacc("TRN2", target_bir_lowering=False)
    # host layout: flat; stored[r, q, i, c] = block r, row 128r+q, img i,
    # col c (with BASE left pad / 4 right cols); +8 elements tail pad so
    # the tile_O DMA (+1 element offset) stays in bounds.
    X = nc.dram_tensor(
        "x", [NBLK * ROWS_BLK * ROW + 8], BF16, kind="ExternalInput"
    )
    WI = nc.dram_tensor("wi", [128, 128], BF16, kind="ExternalInput")
    WNI = nc.dram_tensor("wni", [128, 128], BF16, kind="ExternalInput")
    OUT = nc.dram_tensor("out", [128, 1], F32, kind="ExternalOutput")

    with TileContext(nc) as tc:
        with (
            tc.tile_pool(name="te", bufs=3) as te_pool,
            tc.tile_pool(name="to", bufs=3) as to_pool,
            tc.tile_pool(name="sc", bufs=1) as sc_pool,
            tc.tile_pool(name="acc", bufs=1) as acc_pool,
            tc.psum_pool(name="ps", bufs=2) as ps_pool,
        ):
            stage = acc_pool.tile([128, STAGE_COLS], F32)
            scratch = sc_pool.tile([128, IMGS, W], BF16)
            ascr = acc_pool.tile([128, ACT_GRP * W], BF16)
            wi = acc_pool.tile([128, 128], BF16)
            wni = acc_pool.tile([128, 128], BF16)
            nc.sync.dma_start(out=wi[:], in_=WI[:])
            nc.sync.dma_start(out=wni[:], in_=WNI[:])
            # pre-load the ACT Abs table before any DVE critical section
            # (the lazy table-load DMA deadlocks against critical branches)
            nc.scalar.activation(
                out=ascr[:, 0:2],
                in_=wi[:, 0:2],
                func=mybir.ActivationFunctionType.Abs,
            )
            for r in range(NBLK):
                te = te_pool.tile([128, IMGS, WP], BF16, tag="te")
                to = to_pool.tile([128, 4, IMGS, WP], BF16, tag="to")
                eng = nc.sync if r % 2 == 0 else nc.scalar
                eng.dma_start(
                    out=te[:],
                    in_=bass.AP(X, r * ROWS_BLK * ROW, [[ROW, 128], [1, ROW]]),
                )
                eng.dma_start(
                    out=to[:],
                    in_=bass.AP(
                        X, r * ROWS_BLK * ROW + 1, [[ROW, 128], [1, 4 * ROW]]
                    ),
                )
                base = te[:, :, BASE : BASE + W]

                def bview(j, delta):
                    c0 = BASE + delta - 1
                    return to[:, j, :, c0 : c0 + W]

                # --- PE + ACT: shifts (j, l) for l in {2, 3} ---
                diffs = [(j, l, i) for (j, l) in ACT_SHIFTS for i in range(IMGS)]
                MMW = 512  # matmul free-dim cap (one PSUM bank)
                for g in range(NGRP):
                    psum = ps_pool.tile([128, ACT_GRP * W], F32, tag="ps")
                    # all +I passes first, then all -I (amortize weight loads)
                    for m in range(ACT_GRP):
                        j, l, i = diffs[g * ACT_GRP + m]
                        for c0 in range(0, W, MMW):
                            nc.tensor.matmul(
                                out=psum[:, m * W + c0 : m * W + c0 + MMW],
                                lhsT=wi[:],
                                rhs=te[:, i, BASE + c0 : BASE + c0 + MMW],
                                start=True,
                                stop=False,
                            )
                    for m in range(ACT_GRP):
                        j, l, i = diffs[g * ACT_GRP + m]
                        b0 = BASE + l - 1
                        for c0 in range(0, W, MMW):
                            nc.tensor.matmul(
                                out=psum[:, m * W + c0 : m * W + c0 + MMW],
                                lhsT=wni[:],
                                rhs=to[:, j, i, b0 + c0 : b0 + c0 + MMW],
                                start=False,
                                stop=True,
                            )
                    col = r * (1 + NGRP) + 1 + g
                    nc.scalar.activation(
                        out=ascr[:],
                        in_=psum[:],
                        func=mybir.ActivationFunctionType.Abs,
                        accum_out=stage[:, col : col + 1],
                    )

                # --- DVE chain: 11 cross instructions + read ---
                # No tile_critical: DVE is a sequential queue and the chain
                # is kept contiguous by WAW/WAR deps on the shared scratch
                # tile (every chain op writes scratch; the read reads it).
                for n, (var, j, delta) in enumerate(DVE_SCHED):
                    kind = "seed" if n == 0 else "cont"
                    nc.vector._custom_dve(
                        ops[(kind, var)],
                        out=scratch[:],
                        in0=base,
                        in1=bview(j, delta),
                    ).ins.perf_max = 1
                col = r * (1 + NGRP)
                nc.vector._custom_dve(
                    ops["read"],
                    out=stage[:, col : col + 1],
                    in0=scratch[:, 0:1, 0:1],
                )
            part = acc_pool.tile([128, 1], F32)
            nc.vector.tensor_reduce(
                out=part[:],
                in_=stage[:],
                axis=mybir.AxisListType.X,
                op=mybir.AluOpType.add,
            )
            nc.sync.dma_start(out=OUT[:], in_=part[:])
    return nc


_NC = None


def _get_nc():
    global _NC
    if _NC is None:
        _NC = _build_nc()
        if not _NC.is_finalized():
            _NC.finalize()
    return _NC


def _prep_shards(x: np.ndarray) -> list[dict[str, np.ndarray]]:
    """bf16-cast, circular pad, blockify into the flat per-core layout."""
    imgs = np.ascontiguousarray(x.reshape(B * C, H, W), dtype=np.float32)

    def to_bf16(a32):
        b = a32.view(np.uint32)
        return ((b + 0x7FFF + ((b >> 16) & 1)) >> 16).astype(np.uint16)

    imgs_b = to_bf16(imgs)  # (24, H, W) uint16 view of bf16
    HPAD = H + 3
    even = np.zeros((B * C, HPAD, WP), dtype=np.uint16)
    even[:, :H, BASE : BASE + W] = imgs_b
    even[:, :H, :BASE] = imgs_b[:, :, W - BASE :]
    even[:, :H, BASE + W : BASE + W + 3] = imgs_b[:, :, :3]
    even[:, H:, :] = even[:, :3, :]

    I = np.eye(128, dtype=np.float32)
    wi = to_bf16(I)
    wni = to_bf16(-I)

    shards = even.reshape(NCORES, IMGS, HPAD, WP)
    out = []
    for n in range(NCORES):
        t = shards[n].transpose(1, 0, 2)  # (HPAD, IMGS, WP)
        blk = np.empty((NBLK, ROWS_BLK, IMGS, WP), dtype=np.uint16)
        for r in range(NBLK):
            blk[r] = t[r * RB : r * RB + ROWS_BLK]
        flat = np.concatenate([blk.reshape(-1), np.zeros(8, np.uint16)])
        out.append({"x": flat, "wi": wi, "wni": wni})
    return out


def _run(x: np.ndarray, trace: bool = False):
    import ml_dtypes

    nc = _get_nc()
    in_maps = _prep_shards(x)
    in_maps = [
        {k: v.view(ml_dtypes.bfloat16) for k, v in m.items()} for m in in_maps
    ]
    res = run_bass_kernel_spmd(
        nc, in_maps, core_ids=list(range(NCORES)), trace=trace
    )
    total = 0.0
    for r in res.results:
        total += r["out"].astype(np.float64).sum()
    val = WEIGHT * 2.0 * total / float(B * C * H * W)
    return np.float32(val), res


def kernel(x: np.ndarray) -> np.ndarray:
    x = np.asarray(x, dtype=np.float32)
    val, _ = _run(x, trace=False)
    return val


# revision 6
# speedup vs baseline: 2.2915x; 1.0359x over previous
"""BTV loss kernel for Trainium2 (8 NeuronCores, Bass/Tile).

reference: total = sum over 7x7 neighborhood shifts (k,l) != (0,0) of
           sqrt((x - roll(x,(k,l),axis=(2,3)))**2 + 1e-6).sum()
           out = 0.1 * total / x.size

Math:
  - circular-shift symmetry: shift (k,l) ~ (-k,-l); compute the 24
    half-space shifts {k>0, any l} u {k==0, l>0} and double.
  - sqrt(d^2 + 1e-6) ~= |d| (rel err ~3e-6); bf16 inputs add ~1e-5.

Engines (per 128-row block, 3 images per core):
  - DVE: custom CROSS op: one 2x instruction reads packed pairs of the
    base stream a and a shifted stream b (offset d) and accumulates
      |a_lo-b_lo| (shift d, even cols)   |a_hi-b_hi| (shift d, odd cols)
      |a_lo-b_hi| (shift d+1, even)      |a_hi-b_lo| (shift d-1, odd)
    i.e. 4 absdiff terms/cycle (2x the stock tensor_tensor rate). An
    11-instruction schedule covers shifts {k: l in -3..1} + (0,1..3)
    exactly (nd3/nd4 variants omit one cross term to avoid overlap).
  - PE+ACT: shifts (k,l) for k in 1..3, l in 2,3: PE computes
    diff = base - shift into PSUM via +I/-I matmuls; ACT does
    Abs + accum_out. 18 image-diffs/block in 9 pipelined groups of 2.
  - DMA: per block, tile_E (rows p, phase 0) and tile_O (rows p+j,
    j=0..3, columns shifted +1) are loaded straight from HBM; all
    odd column offsets come from tile_O so every DVE stream stays
    4-byte aligned with no on-chip repacking.

Distribution: pure data parallel over the 24 (b,c) images, 3 per core;
host sums the 8 per-core partials in f64.
"""

import dataclasses
import re
from operator import add as _py_add

import numpy as np

import concourse.bass as bass
import concourse.bacc as bacc_mod
import concourse.mybir as mybir
from concourse import dve_ops as _dvo
from concourse.dve_spec import AluOp as _DveAluOp
from concourse.dve_spec import Bin, Spec, Src0, Src1
from concourse.tile import TileContext
from concourse.bass_utils import run_bass_kernel_spmd

from concourse.dve_uop import (
    ENABLE,
    AluInp,
    AluOp as UAluOp,
    DelayInp,
    InpSel,
    OutPath,
    OutSel,
    Trigger,
    UopConfig,
)

B, C, H, W = 8, 3, 1024, 1024
NCORES = 8
IMGS = (B * C) // NCORES        # images per core = 3
BASE = 4                        # left col pad (even => 4B-aligned in bf16)
WP = W + BASE + 3 + 1           # 1032: [w-4..w-1][0..1023][0,1,2][pad]
RB = 128                        # rows per block (partition dim)
NBLK = H // RB                  # 8 row blocks per image
ROWS_BLK = RB + 3               # 131 rows stored per block (128 + 3 halo)
ROW = IMGS * WP                 # elements per stored row

WEIGHT = 0.1
F32 = mybir.dt.float32
BF16 = mybir.dt.bfloat16

# DVE schedule: (kind, variant, j=row phase, delta). Covers, per j>0,
# l in {-3..1} (ACT takes l=2,3), and for j=0 l in {1,2,3}:
#   full: F(d)+E(d+1)+O(d-1);  nd3: F(d)+O(d-1);  nd4: F(d)+E(d+1)
DVE_SCHED = [
    ("full", 0, 1),   # F(1), E(2), O(0)=0      [k=0]
    ("nd3", 0, 3),    # F(3), O(2)              [k=0]
]
for _j in (1, 2, 3):
    DVE_SCHED += [
        ("nd4", _j, -3),   # F(-3), E(-2)
        ("full", _j, -1),  # F(-1), E(0), O(-2)
        ("nd3", _j, 1),    # F(1), O(0)
    ]
# PE/ACT shifts: (j, l) pairs
ACT_SHIFTS = [(j, l) for j in (1, 2, 3) for l in (2, 3)]
ACT_GRP = 2  # image-diffs per PSUM tile / ACT instruction (4 PSUM banks)


def _mk_cross_uop(kind: str, use_d3: bool, use_d4: bool):
    """2x CROSS uop. kind: "seed" (acc <- sum on elem 0) | "steady".
    blocks: 0:d1  1:d4  2:d3  3:d2  4..6:sum tree  7:acc
    Omitted terms compute ABSDIFF(x, x) = 0 instead (same structure).
    """
    u = UopConfig()
    u.enable_input(InpSel.SRC_0, 0)      # a_lo -> ALU lane
    u.enable_input(InpSel.SRC_1, 1)      # b_lo -> delay lane 0
    u.enable_input(InpSel.SRC_0_HI, 2)   # a_hi -> delay lane 1
    u.enable_input(InpSel.SRC_1_HI, 3)   # b_hi -> delay lane 2
    u.accum_enabled = ENABLE
    dp = u.datapath_config
    dp[0].enable_alu(UAluOp.ABSOLUTE_DIFF, AluInp.PREV_ALU_OUT, AluInp.PREV_DELAY_0)
    dp[0].enable_delay_from_src(DelayInp.PREV_ALU_OUT, 3)
    dp[0].pass_through_delay(0, 1, 2)
    dp[1].enable_alu(
        UAluOp.ABSOLUTE_DIFF,
        AluInp.PREV_DELAY_1,
        AluInp.PREV_DELAY_0 if use_d4 else AluInp.PREV_DELAY_1,
    )
    dp[1].enable_delay_from_src(DelayInp.PREV_ALU_OUT, 0)
    dp[1].pass_through_delay(1, 2, 3)
    dp[2].enable_alu(
        UAluOp.ABSOLUTE_DIFF,
        AluInp.PREV_DELAY_3,
        AluInp.PREV_DELAY_2 if use_d3 else AluInp.PREV_DELAY_3,
    )
    dp[2].enable_delay_from_src(DelayInp.PREV_ALU_OUT, 3)
    dp[2].pass_through_delay(0, 1, 2)
    dp[3].enable_alu(UAluOp.ABSOLUTE_DIFF, AluInp.PREV_DELAY_1, AluInp.PREV_DELAY_2)
    dp[3].enable_delay_from_src(DelayInp.PREV_ALU_OUT, 1)
    dp[3].pass_through_delay(0, 3)
    dp[4].enable_alu(UAluOp.ADD, AluInp.PREV_ALU_OUT, AluInp.PREV_DELAY_1)
    dp[4].pass_through_delay(0, 3)
    dp[5].enable_alu(UAluOp.ADD, AluInp.PREV_ALU_OUT, AluInp.PREV_DELAY_0)
    dp[5].pass_through_delay(3)
    dp[6].enable_alu(UAluOp.ADD, AluInp.PREV_ALU_OUT, AluInp.PREV_DELAY_3)
    if kind == "seed":
        dp[7].enable_alu(UAluOp.BYPASS, AluInp.PREV_ALU_OUT, AluInp.PREV_ALU_OUT)
    else:
        dp[7].enable_alu(UAluOp.ADD, AluInp.CURR_ALU_OUT, AluInp.PREV_ALU_OUT)
    dp[7].alu_out_a_enable = ENABLE
    u.require_inp0 = ENABLE
    u.require_inp1 = ENABLE
    u.enable_output(OutSel.ALU_OUT, OutPath.WR0_LO)
    u.enable_output(OutSel.ALU_OUT, OutPath.WR0_HI)
    if kind == "seed":
        u.trigger = (Trigger.COUNT, Trigger.SRC_TENSOR_DONE, Trigger.NONE)
        u.next_uop = (1, 0, 0)
        u.repeat_count = 1
    else:
        u.trigger = (Trigger.SRC_TENSOR_DONE, Trigger.NONE, Trigger.NONE)
        u.next_uop = (0, 0, 0)
    return u


def _mk_poison_1x():
    """1x fallback: acc <- +inf so any non-2x execution is caught."""
    u = UopConfig()
    u.enable_input(InpSel.SRC_0, 0)
    u.enable_input(InpSel.POS_INF, 1)
    dp = u.datapath_config
    for b in range(7):
        dp[b].enable_alu(UAluOp.BYPASS, AluInp.PREV_ALU_OUT, AluInp.PREV_ALU_OUT)
        dp[b].pass_through_delay(0)
    dp[7].enable_alu(UAluOp.BYPASS, AluInp.PREV_DELAY_0, AluInp.PREV_DELAY_0)
    dp[7].alu_out_a_enable = ENABLE
    u.accum_enabled = ENABLE
    u.require_inp0 = ENABLE
    u.require_inp1 = ENABLE
    u.enable_output(OutSel.ALU_OUT, OutPath.WR0_LO)
    u.trigger = (Trigger.SRC_TENSOR_DONE, Trigger.NONE, Trigger.NONE)
    u.next_uop = (0, 0, 0)
    return u


def _mk_read_uop():
    """Route blk7's accumulator flop to the output (1-element stream)."""
    u = UopConfig()
    u.enable_input(InpSel.SRC_0, 0)
    dp = u.datapath_config
    for b in range(7):
        dp[b].enable_alu(UAluOp.BYPASS, AluInp.PREV_ALU_OUT, AluInp.PREV_ALU_OUT)
    dp[7].enable_alu(UAluOp.BYPASS, AluInp.CURR_ALU_OUT, AluInp.CURR_ALU_OUT)
    u.require_inp0 = ENABLE
    u.enable_output(OutSel.ALU_OUT, OutPath.WR0_LO)
    u.trigger = (Trigger.SRC_TENSOR_DONE, Trigger.NONE, Trigger.NONE)
    u.next_uop = (0, 0, 0)
    return u


class _HandDveOp(_dvo.DveOp):
    BUILDERS = {}  # name -> (build_1x_list, build_2x_list_or_None, rd1_en)

    def compile(self, ver):
        from concourse.dve_uop import DveOpSpec

        key = (self.name, ver)
        if (r := _dvo._COMPILE_CACHE.get(key)) is not None:
            return r
        b1, b2, rd1 = self.BUILDERS[self.name]
        result = DveOpSpec(
            name=self.name,
            opcode=_dvo.get_dve_sub_opcode(self.name),
            uops=b1(),
            uops_2x=(b2() if b2 is not None else None),
            rd1_en=rd1,
        )
        got = result.sha(ver)
        if self.uops_sha.get(ver) != got:
            raise ValueError(f"sha drift ({ver}: {got} != pinned)")
        _dvo._COMPILE_CACHE[key] = result
        return result


def _register(name, spec, build_1x, build_2x, rd1_en):
    _HandDveOp.BUILDERS[name] = (build_1x, build_2x, rd1_en)
    op = _HandDveOp(name, spec, subdim=False, uops_sha={})
    _dvo._SUB_OPCODE_FOR_NAME[name] = _dvo._CUSTOM_DVE_ROW_BASE + len(_dvo.OPS)
    shas = {}
    for ver in ("v3", "v4"):
        try:
            op.compile(ver)
            shas[ver] = op.uops_sha.get(ver)
        except ValueError as e:
            m = re.search(r"([0-9a-f]{16})", str(e))
            if not m:
                raise
            shas[ver] = m.group(1)
    op = dataclasses.replace(op, uops_sha=shas)
    _dvo.OPS.append(op)
    _dvo.CUSTOM_DVE_SPECS[name] = spec
    return op


_OPS = None


def _get_ops():
    """dict: (kind, variant) -> op, plus 'read'."""
    global _OPS
    if _OPS is not None:
        return _OPS
    have = {op.name: op for op in _dvo.OPS}
    names = {
        ("seed", "full"): "XR_SEED_F_ANT",
        ("seed", "nd3"): "XR_SEED_ND3_ANT",
        ("seed", "nd4"): "XR_SEED_ND4_ANT",
        ("cont", "full"): "XR_CONT_F_ANT",
        ("cont", "nd3"): "XR_CONT_ND3_ANT",
        ("cont", "nd4"): "XR_CONT_ND4_ANT",
    }
    if names[("seed", "full")] in have:
        _OPS = {k: have[n] for k, n in names.items()}
        _OPS["read"] = have["XR_READ_ANT"]
        return _OPS

    def _ref(in0, in1, s0, s1, imm2):
        a = in0.astype(np.float32)
        b = in1.astype(np.float32)
        P = a.shape[0]
        out = np.abs(a.reshape(P, -1) - b.reshape(P, -1))
        return out.reshape(in0.shape), out.reshape(P, -1).sum(-1, keepdims=True)

    spec_acc = Spec(
        body=Bin(_DveAluOp.ABSOLUTE_DIFF, Src0, Src1),
        accum=_py_add,
        reference=_ref,
    )
    spec_read = Spec(
        body=Src0,
        reference=lambda in0, in1, s0, s1, imm2: in0.astype(np.float32),
    )
    _OPS = {}
    for (kind, var), name in names.items():
        d3, d4 = var != "nd3", var != "nd4"
        _OPS[(kind, var)] = _register(
            name,
            spec_acc,
            lambda: [_mk_poison_1x(), _mk_poison_1x()],
            lambda kind=kind, d3=d3, d4=d4: [
                _mk_cross_uop(kind, d3, d4),
                _mk_cross_uop("steady", d3, d4),
            ],
            True,
        )
    _OPS["read"] = _register(
        "XR_READ_ANT", spec_read, lambda: [_mk_read_uop()], None, False
    )
    return _OPS


NGRP = (len(ACT_SHIFTS) * IMGS) // ACT_GRP  # ACT groups per block = 9
STAGE_COLS = NBLK * (1 + NGRP)              # 80


def _build_nc():
    ops = _get_ops()
    nc = bacc_mod.Bacc("TRN2", target_bir_lowering=False)
    # host layout: flat; stored[r, q, i, c] = block r, row 128r+q, img i,
    # col c (with BASE left pad / 4 right cols); +8 elements tail pad so
    # the tile_O DMA (+1 element offset) stays in bounds.
    X = nc.dram_tensor(
        "x", [NBLK * ROWS_BLK * ROW + 8], BF16, kind="ExternalInput"
    )
    WI = nc.dram_tensor("wi", [128, 128], BF16, kind="ExternalInput")
    WNI = nc.dram_tensor("wni", [128, 128], BF16, kind="ExternalInput")
    OUT = nc.dram_tensor("out", [128, 1], F32, kind="ExternalOutput")

    with TileContext(nc) as tc:
        with (
            tc.tile_pool(name="te", bufs=3) as te_pool,
            tc.tile_pool(name="to", bufs=3) as to_pool,
            tc.tile_pool(name="sc", bufs=1) as sc_pool,
            tc.tile_pool(name="acc", bufs=1) as acc_pool,
            tc.psum_pool(name="ps", bufs=2) as ps_pool,
        ):
            stage = acc_pool.tile([128, STAGE_COLS], F32)
            scratch = sc_pool.tile([128, IMGS, W], BF16)
            ascr = acc_pool.tile([128, ACT_GRP * W], BF16)
            wi = acc_pool.tile([128, 128], BF16)
            wni = acc_pool.tile([128, 128], BF16)
            nc.sync.dma_start(out=wi[:], in_=WI[:])
            nc.sync.dma_start(out=wni[:], in_=WNI[:])
            # pre-load the ACT Abs table before any DVE critical section
            # (the lazy table-load DMA deadlocks against critical branches)
            nc.scalar.activation(
                out=ascr[:, 0:2],
                in_=wi[:, 0:2],
                func=mybir.ActivationFunctionType.Abs,
            )
            for r in range(NBLK):
                te = te_pool.tile([128, IMGS, WP], BF16, tag="te")
                to = to_pool.tile([128, 4, IMGS, WP], BF16, tag="to")
                nc.scalar.dma_start(
                    out=te[:],
                    in_=bass.AP(X, r * ROWS_BLK * ROW, [[ROW, 128], [1, ROW]]),
                )
                nc.sync.dma_start(
                    out=to[:],
                    in_=bass.AP(
                        X, r * ROWS_BLK * ROW + 1, [[ROW, 128], [1, 4 * ROW]]
                    ),
                )
                base = te[:, :, BASE : BASE + W]

                def bview(j, delta):
                    c0 = BASE + delta - 1
                    return to[:, j, :, c0 : c0 + W]

                # --- PE + ACT: shifts (j, l) for l in {2, 3} ---
                diffs = [(j, l, i) for (j, l) in ACT_SHIFTS for i in range(IMGS)]
                MMW = 512  # matmul free-dim cap (one PSUM bank)
                for g in range(NGRP):
                    psum = ps_pool.tile([128, ACT_GRP * W], F32, tag="ps")
                    # all +I passes first, then all -I (amortize weight loads)
                    for m in range(ACT_GRP):
                        j, l, i = diffs[g * ACT_GRP + m]
                        for c0 in range(0, W, MMW):
                            nc.tensor.matmul(
                                out=psum[:, m * W + c0 : m * W + c0 + MMW],
                                lhsT=wi[:],
                                rhs=te[:, i, BASE + c0 : BASE + c0 + MMW],
                                start=True,
                                stop=False,
                            )
                    for m in range(ACT_GRP):
                        j, l, i = diffs[g * ACT_GRP + m]
                        b0 = BASE + l - 1
                        for c0 in range(0, W, MMW):
                            nc.tensor.matmul(
                                out=psum[:, m * W + c0 : m * W + c0 + MMW],
                                lhsT=wni[:],
                                rhs=to[:, j, i, b0 + c0 : b0 + c0 + MMW],
                                start=False,
                                stop=True,
                            )
                    col = r * (1 + NGRP) + 1 + g
                    nc.scalar.activation(
                        out=ascr[:],
                        in_=psum[:],
                        func=mybir.ActivationFunctionType.Abs,
                        accum_out=stage[:, col : col + 1],
                    )

                # --- DVE chain: 11 cross instructions + read ---
                # No tile_critical: DVE is a sequential queue and the chain
                # is kept contiguous by WAW/WAR deps on the shared scratch
                # tile (every chain op writes scratch; the read reads it).
                for n, (var, j, delta) in enumerate(DVE_SCHED):
                    kind = "seed" if n == 0 else "cont"
                    nc.vector._custom_dve(
                        ops[(kind, var)],
                        out=scratch[:],
                        in0=base,
                        in1=bview(j, delta),
                    ).ins.perf_max = 1
                col = r * (1 + NGRP)
                nc.vector._custom_dve(
                    ops["read"],
                    out=stage[:, col : col + 1],
                    in0=scratch[:, 0:1, 0:1],
                )
            part = acc_pool.tile([128, 1], F32)
            nc.vector.tensor_reduce(
                out=part[:],
                in_=stage[:],
                axis=mybir.AxisListType.X,
                op=mybir.AluOpType.add,
            )
            nc.sync.dma_start(out=OUT[:], in_=part[:])
    return nc


_NC = None


def _get_nc():
    global _NC
    if _NC is None:
        _NC = _build_nc()
        if not _NC.is_finalized():
            _NC.finalize()
    return _NC


def _prep_shards(x: np.ndarray) -> list[dict[str, np.ndarray]]:
    """bf16-cast, circular pad, blockify into the flat per-core layout."""
    imgs = np.ascontiguousarray(x.reshape(B * C, H, W), dtype=np.float32)

    def to_bf16(a32):
        b = a32.view(np.uint32)
        return ((b + 0x7FFF + ((b >> 16) & 1)) >> 16).astype(np.uint16)

    imgs_b = to_bf16(imgs)  # (24, H, W) uint16 view of bf16
    HPAD = H + 3
    even = np.zeros((B * C, HPAD, WP), dtype=np.uint16)
    even[:, :H, BASE : BASE + W] = imgs_b
    even[:, :H, :BASE] = imgs_b[:, :, W - BASE :]
    even[:, :H, BASE + W : BASE + W + 3] = imgs_b[:, :, :3]
    even[:, H:, :] = even[:, :3, :]

    I = np.eye(128, dtype=np.float32)
    wi = to_bf16(I)
    wni = to_bf16(-I)

    shards = even.reshape(NCORES, IMGS, HPAD, WP)
    out = []
    for n in range(NCORES):
        t = shards[n].transpose(1, 0, 2)  # (HPAD, IMGS, WP)
        blk = np.empty((NBLK, ROWS_BLK, IMGS, WP), dtype=np.uint16)
        for r in range(NBLK):
            blk[r] = t[r * RB : r * RB + ROWS_BLK]
        flat = np.concatenate([blk.reshape(-1), np.zeros(8, np.uint16)])
        out.append({"x": flat, "wi": wi, "wni": wni})
    return out


def _run(x: np.ndarray, trace: bool = False):
    import ml_dtypes

    nc = _get_nc()
    in_maps = _prep_shards(x)
    in_maps = [
        {k: v.view(ml_dtypes.bfloat16) for k, v in m.items()} for m in in_maps
    ]
    res = run_bass_kernel_spmd(
        nc, in_maps, core_ids=list(range(NCORES)), trace=trace
    )
    total = 0.0
    for r in res.results:
        total += r["out"].astype(np.float64).sum()
    val = WEIGHT * 2.0 * total / float(B * C * H * W)
    return np.float32(val), res


def kernel(x: np.ndarray) -> np.ndarray:
    x = np.asarray(x, dtype=np.float32)
    val, _ = _run(x, trace=False)
    return val


# revision 7
# speedup vs baseline: 2.3011x; 1.0042x over previous
"""BTV loss kernel for Trainium2 (8 NeuronCores, Bass/Tile).

reference: total = sum over 7x7 neighborhood shifts (k,l) != (0,0) of
           sqrt((x - roll(x,(k,l),axis=(2,3)))**2 + 1e-6).sum()
           out = 0.1 * total / x.size

Math:
  - circular-shift symmetry: shift (k,l) ~ (-k,-l); compute the 24
    half-space shifts {k>0, any l} u {k==0, l>0} and double.
  - sqrt(d^2 + 1e-6) ~= |d| (rel err ~3e-6); bf16 inputs add ~1e-5.

Engines (per 128-row block, 3 images per core):
  - DVE: custom CROSS op: one 2x instruction reads packed pairs of the
    base stream a and a shifted stream b (offset d) and accumulates
      |a_lo-b_lo| (shift d, even cols)   |a_hi-b_hi| (shift d, odd cols)
      |a_lo-b_hi| (shift d+1, even)      |a_hi-b_lo| (shift d-1, odd)
    i.e. 4 absdiff terms/cycle (2x the stock tensor_tensor rate). An
    11-instruction schedule covers shifts {k: l in -3..1} + (0,1..3)
    exactly (nd3/nd4 variants omit one cross term to avoid overlap).
  - PE+ACT: shifts (k,l) for k in 1..3, l in 2,3: PE computes
    diff = base - shift into PSUM via +I/-I matmuls; ACT does
    Abs + accum_out. 18 image-diffs/block in 9 pipelined groups of 2.
  - DMA: per block, tile_E (rows p, phase 0) and tile_O (rows p+j,
    j=0..3, columns shifted +1) are loaded straight from HBM; all
    odd column offsets come from tile_O so every DVE stream stays
    4-byte aligned with no on-chip repacking.

Distribution: pure data parallel over the 24 (b,c) images, 3 per core;
host sums the 8 per-core partials in f64.
"""

import dataclasses
import re
from operator import add as _py_add

import numpy as np

import concourse.bass as bass
import concourse.bacc as bacc_mod
import concourse.mybir as mybir
from concourse import dve_ops as _dvo
from concourse.dve_spec import AluOp as _DveAluOp
from concourse.dve_spec import Bin, Spec, Src0, Src1
from concourse.tile import TileContext
from concourse.bass_utils import run_bass_kernel_spmd

from concourse.dve_uop import (
    ENABLE,
    AluInp,
    AluOp as UAluOp,
    DelayInp,
    InpSel,
    OutPath,
    OutSel,
    Trigger,
    UopConfig,
)

B, C, H, W = 8, 3, 1024, 1024
NCORES = 8
IMGS = (B * C) // NCORES        # images per core = 3
BASE = 4                        # left col pad (even => 4B-aligned in bf16)
WP = W + BASE + 3 + 1           # 1032: [w-4..w-1][0..1023][0,1,2][pad]
RB = 128                        # rows per block (partition dim)
NBLK = H // RB                  # 8 row blocks per image
ROWS_BLK = RB + 3               # 131 rows stored per block (128 + 3 halo)
ROW = IMGS * WP                 # elements per stored row

WEIGHT = 0.1
F32 = mybir.dt.float32
BF16 = mybir.dt.bfloat16

# DVE schedule: (kind, variant, j=row phase, delta). Covers, per j>0,
# l in {-3..1} (ACT takes l=2,3), and for j=0 l in {1,2,3}:
#   full: F(d)+E(d+1)+O(d-1);  nd3: F(d)+O(d-1);  nd4: F(d)+E(d+1)
DVE_SCHED = [
    ("full", 0, 1),   # F(1), E(2), O(0)=0      [k=0]
    ("nd3", 0, 3),    # F(3), O(2)              [k=0]
]
for _j in (1, 2, 3):
    DVE_SCHED += [
        ("nd4", _j, -3),   # F(-3), E(-2)
        ("full", _j, -1),  # F(-1), E(0), O(-2)
        ("nd3", _j, 1),    # F(1), O(0)
    ]
# PE/ACT shifts: (j, l) pairs
ACT_SHIFTS = [(j, l) for j in (1, 2, 3) for l in (2, 3)]
ACT_GRP = 2  # image-diffs per PSUM tile / ACT instruction (4 PSUM banks)


def _mk_cross_uop(kind: str, use_d3: bool, use_d4: bool):
    """2x CROSS uop. kind: "seed" (acc <- sum on elem 0) | "steady".
    blocks: 0:d1  1:d4  2:d3  3:d2  4..6:sum tree  7:acc
    Omitted terms compute ABSDIFF(x, x) = 0 instead (same structure).
    """
    u = UopConfig()
    u.enable_input(InpSel.SRC_0, 0)      # a_lo -> ALU lane
    u.enable_input(InpSel.SRC_1, 1)      # b_lo -> delay lane 0
    u.enable_input(InpSel.SRC_0_HI, 2)   # a_hi -> delay lane 1
    u.enable_input(InpSel.SRC_1_HI, 3)   # b_hi -> delay lane 2
    u.accum_enabled = ENABLE
    dp = u.datapath_config
    dp[0].enable_alu(UAluOp.ABSOLUTE_DIFF, AluInp.PREV_ALU_OUT, AluInp.PREV_DELAY_0)
    dp[0].enable_delay_from_src(DelayInp.PREV_ALU_OUT, 3)
    dp[0].pass_through_delay(0, 1, 2)
    dp[1].enable_alu(
        UAluOp.ABSOLUTE_DIFF,
        AluInp.PREV_DELAY_1,
        AluInp.PREV_DELAY_0 if use_d4 else AluInp.PREV_DELAY_1,
    )
    dp[1].enable_delay_from_src(DelayInp.PREV_ALU_OUT, 0)
    dp[1].pass_through_delay(1, 2, 3)
    dp[2].enable_alu(
        UAluOp.ABSOLUTE_DIFF,
        AluInp.PREV_DELAY_3,
        AluInp.PREV_DELAY_2 if use_d3 else AluInp.PREV_DELAY_3,
    )
    dp[2].enable_delay_from_src(DelayInp.PREV_ALU_OUT, 3)
    dp[2].pass_through_delay(0, 1, 2)
    dp[3].enable_alu(UAluOp.ABSOLUTE_DIFF, AluInp.PREV_DELAY_1, AluInp.PREV_DELAY_2)
    dp[3].enable_delay_from_src(DelayInp.PREV_ALU_OUT, 1)
    dp[3].pass_through_delay(0, 3)
    dp[4].enable_alu(UAluOp.ADD, AluInp.PREV_ALU_OUT, AluInp.PREV_DELAY_1)
    dp[4].pass_through_delay(0, 3)
    dp[5].enable_alu(UAluOp.ADD, AluInp.PREV_ALU_OUT, AluInp.PREV_DELAY_0)
    dp[5].pass_through_delay(3)
    dp[6].enable_alu(UAluOp.ADD, AluInp.PREV_ALU_OUT, AluInp.PREV_DELAY_3)
    if kind == "seed":
        dp[7].enable_alu(UAluOp.BYPASS, AluInp.PREV_ALU_OUT, AluInp.PREV_ALU_OUT)
    else:
        dp[7].enable_alu(UAluOp.ADD, AluInp.CURR_ALU_OUT, AluInp.PREV_ALU_OUT)
    dp[7].alu_out_a_enable = ENABLE
    u.require_inp0 = ENABLE
    u.require_inp1 = ENABLE
    u.enable_output(OutSel.ALU_OUT, OutPath.WR0_LO)
    u.enable_output(OutSel.ALU_OUT, OutPath.WR0_HI)
    if kind == "seed":
        u.trigger = (Trigger.COUNT, Trigger.SRC_TENSOR_DONE, Trigger.NONE)
        u.next_uop = (1, 0, 0)
        u.repeat_count = 1
    else:
        u.trigger = (Trigger.SRC_TENSOR_DONE, Trigger.NONE, Trigger.NONE)
        u.next_uop = (0, 0, 0)
    return u


def _mk_poison_1x():
    """1x fallback: acc <- +inf so any non-2x execution is caught."""
    u = UopConfig()
    u.enable_input(InpSel.SRC_0, 0)
    u.enable_input(InpSel.POS_INF, 1)
    dp = u.datapath_config
    for b in range(7):
        dp[b].enable_alu(UAluOp.BYPASS, AluInp.PREV_ALU_OUT, AluInp.PREV_ALU_OUT)
        dp[b].pass_through_delay(0)
    dp[7].enable_alu(UAluOp.BYPASS, AluInp.PREV_DELAY_0, AluInp.PREV_DELAY_0)
    dp[7].alu_out_a_enable = ENABLE
    u.accum_enabled = ENABLE
    u.require_inp0 = ENABLE
    u.require_inp1 = ENABLE
    u.enable_output(OutSel.ALU_OUT, OutPath.WR0_LO)
    u.trigger = (Trigger.SRC_TENSOR_DONE, Trigger.NONE, Trigger.NONE)
    u.next_uop = (0, 0, 0)
    return u


def _mk_read_uop():
    """Route blk7's accumulator flop to the output (1-element stream)."""
    u = UopConfig()
    u.enable_input(InpSel.SRC_0, 0)
    dp = u.datapath_config
    for b in range(7):
        dp[b].enable_alu(UAluOp.BYPASS, AluInp.PREV_ALU_OUT, AluInp.PREV_ALU_OUT)
    dp[7].enable_alu(UAluOp.BYPASS, AluInp.CURR_ALU_OUT, AluInp.CURR_ALU_OUT)
    u.require_inp0 = ENABLE
    u.enable_output(OutSel.ALU_OUT, OutPath.WR0_LO)
    u.trigger = (Trigger.SRC_TENSOR_DONE, Trigger.NONE, Trigger.NONE)
    u.next_uop = (0, 0, 0)
    return u


class _HandDveOp(_dvo.DveOp):
    BUILDERS = {}  # name -> (build_1x_list, build_2x_list_or_None, rd1_en)

    def compile(self, ver):
        from concourse.dve_uop import DveOpSpec

        key = (self.name, ver)
        if (r := _dvo._COMPILE_CACHE.get(key)) is not None:
            return r
        b1, b2, rd1 = self.BUILDERS[self.name]
        result = DveOpSpec(
            name=self.name,
            opcode=_dvo.get_dve_sub_opcode(self.name),
            uops=b1(),
            uops_2x=(b2() if b2 is not None else None),
            rd1_en=rd1,
        )
        got = result.sha(ver)
        if self.uops_sha.get(ver) != got:
            raise ValueError(f"sha drift ({ver}: {got} != pinned)")
        _dvo._COMPILE_CACHE[key] = result
        return result


def _register(name, spec, build_1x, build_2x, rd1_en):
    _HandDveOp.BUILDERS[name] = (build_1x, build_2x, rd1_en)
    op = _HandDveOp(name, spec, subdim=False, uops_sha={})
    _dvo._SUB_OPCODE_FOR_NAME[name] = _dvo._CUSTOM_DVE_ROW_BASE + len(_dvo.OPS)
    shas = {}
    for ver in ("v3", "v4"):
        try:
            op.compile(ver)
            shas[ver] = op.uops_sha.get(ver)
        except ValueError as e:
            m = re.search(r"([0-9a-f]{16})", str(e))
            if not m:
                raise
            shas[ver] = m.group(1)
    op = dataclasses.replace(op, uops_sha=shas)
    _dvo.OPS.append(op)
    _dvo.CUSTOM_DVE_SPECS[name] = spec
    return op


_OPS = None


def _get_ops():
    """dict: (kind, variant) -> op, plus 'read'."""
    global _OPS
    if _OPS is not None:
        return _OPS
    have = {op.name: op for op in _dvo.OPS}
    names = {
        ("seed", "full"): "XR_SEED_F_ANT",
        ("seed", "nd3"): "XR_SEED_ND3_ANT",
        ("seed", "nd4"): "XR_SEED_ND4_ANT",
        ("cont", "full"): "XR_CONT_F_ANT",
        ("cont", "nd3"): "XR_CONT_ND3_ANT",
        ("cont", "nd4"): "XR_CONT_ND4_ANT",
    }
    if names[("seed", "full")] in have:
        _OPS = {k: have[n] for k, n in names.items()}
        _OPS["read"] = have["XR_READ_ANT"]
        return _OPS

    def _ref(in0, in1, s0, s1, imm2):
        a = in0.astype(np.float32)
        b = in1.astype(np.float32)
        P = a.shape[0]
        out = np.abs(a.reshape(P, -1) - b.reshape(P, -1))
        return out.reshape(in0.shape), out.reshape(P, -1).sum(-1, keepdims=True)

    spec_acc = Spec(
        body=Bin(_DveAluOp.ABSOLUTE_DIFF, Src0, Src1),
        accum=_py_add,
        reference=_ref,
    )
    spec_read = Spec(
        body=Src0,
        reference=lambda in0, in1, s0, s1, imm2: in0.astype(np.float32),
    )
    _OPS = {}
    for (kind, var), name in names.items():
        d3, d4 = var != "nd3", var != "nd4"
        _OPS[(kind, var)] = _register(
            name,
            spec_acc,
            lambda: [_mk_poison_1x(), _mk_poison_1x()],
            lambda kind=kind, d3=d3, d4=d4: [
                _mk_cross_uop(kind, d3, d4),
                _mk_cross_uop("steady", d3, d4),
            ],
            True,
        )
    _OPS["read"] = _register(
        "XR_READ_ANT", spec_read, lambda: [_mk_read_uop()], None, False
    )
    return _OPS


NGRP = (len(ACT_SHIFTS) * IMGS) // ACT_GRP  # ACT groups per block = 9
STAGE_COLS = NBLK * (1 + NGRP)              # 80


def _build_nc():
    ops = _get_ops()
    nc = bacc_mod.Bacc("TRN2", target_bir_lowering=False)
    # host layout: flat; stored[r, q, i, c] = block r, row 128r+q, img i,
    # col c (with BASE left pad / 4 right cols); +8 elements tail pad so
    # the tile_O DMA (+1 element offset) stays in bounds.
    X = nc.dram_tensor(
        "x", [NBLK * ROWS_BLK * ROW + 8], BF16, kind="ExternalInput"
    )
    WI = nc.dram_tensor("wi", [128, 128], BF16, kind="ExternalInput")
    WNI = nc.dram_tensor("wni", [128, 128], BF16, kind="ExternalInput")
    OUT = nc.dram_tensor("out", [128, 1], F32, kind="ExternalOutput")

    with TileContext(nc) as tc:
        with (
            tc.tile_pool(name="te", bufs=3) as te_pool,
            tc.tile_pool(name="to", bufs=3) as to_pool,
            tc.tile_pool(name="sc", bufs=1) as sc_pool,
            tc.tile_pool(name="acc", bufs=1) as acc_pool,
            tc.psum_pool(name="ps", bufs=2) as ps_pool,
        ):
            stage = acc_pool.tile([128, STAGE_COLS], F32)
            scratch = sc_pool.tile([128, IMGS, W], BF16)
            ascr = acc_pool.tile([128, ACT_GRP * W], BF16)
            wi = acc_pool.tile([128, 128], BF16)
            wni = acc_pool.tile([128, 128], BF16)
            nc.sync.dma_start(out=wi[:], in_=WI[:])
            nc.sync.dma_start(out=wni[:], in_=WNI[:])
            # pre-load the ACT Abs table before any DVE critical section
            # (the lazy table-load DMA deadlocks against critical branches)
            nc.scalar.activation(
                out=ascr[:, 0:2],
                in_=wi[:, 0:2],
                func=mybir.ActivationFunctionType.Abs,
            )
            for r in range(NBLK):
                te = te_pool.tile([128, IMGS, WP], BF16, tag="te")
                to = to_pool.tile([128, 4, IMGS, WP], BF16, tag="to")
                # split across both HWDGE rings; phases 0-1 land first so
                # the k=0/k=1 DVE instructions and j=1 PE groups start early
                nc.scalar.dma_start(
                    out=te[:],
                    in_=bass.AP(X, r * ROWS_BLK * ROW, [[ROW, 128], [1, ROW]]),
                )
                nc.sync.dma_start(
                    out=to[:, 0:2],
                    in_=bass.AP(
                        X, r * ROWS_BLK * ROW + 1, [[ROW, 128], [1, 2 * ROW]]
                    ),
                )
                nc.scalar.dma_start(
                    out=to[:, 2:4],
                    in_=bass.AP(
                        X,
                        r * ROWS_BLK * ROW + 2 * ROW + 1,
                        [[ROW, 128], [1, 2 * ROW]],
                    ),
                )
                base = te[:, :, BASE : BASE + W]

                def bview(j, delta):
                    c0 = BASE + delta - 1
                    return to[:, j, :, c0 : c0 + W]

                # --- PE + ACT: shifts (j, l) for l in {2, 3} ---
                diffs = [(j, l, i) for (j, l) in ACT_SHIFTS for i in range(IMGS)]
                MMW = 512  # matmul free-dim cap (one PSUM bank)
                for g in range(NGRP):
                    psum = ps_pool.tile([128, ACT_GRP * W], F32, tag="ps")
                    # all +I passes first, then all -I (amortize weight loads)
                    for m in range(ACT_GRP):
                        j, l, i = diffs[g * ACT_GRP + m]
                        for c0 in range(0, W, MMW):
                            nc.tensor.matmul(
                                out=psum[:, m * W + c0 : m * W + c0 + MMW],
                                lhsT=wi[:],
                                rhs=te[:, i, BASE + c0 : BASE + c0 + MMW],
                                start=True,
                                stop=False,
                            )
                    for m in range(ACT_GRP):
                        j, l, i = diffs[g * ACT_GRP + m]
                        b0 = BASE + l - 1
                        for c0 in range(0, W, MMW):
                            nc.tensor.matmul(
                                out=psum[:, m * W + c0 : m * W + c0 + MMW],
                                lhsT=wni[:],
                                rhs=to[:, j, i, b0 + c0 : b0 + c0 + MMW],
                                start=False,
                                stop=True,
                            )
                    col = r * (1 + NGRP) + 1 + g
                    nc.scalar.activation(
                        out=ascr[:],
                        in_=psum[:],
                        func=mybir.ActivationFunctionType.Abs,
                        accum_out=stage[:, col : col + 1],
                    )

                # --- DVE chain: 11 cross instructions + read ---
                # No tile_critical: DVE is a sequential queue and the chain
                # is kept contiguous by WAW/WAR deps on the shared scratch
                # tile (every chain op writes scratch; the read reads it).
                for n, (var, j, delta) in enumerate(DVE_SCHED):
                    kind = "seed" if n == 0 else "cont"
                    nc.vector._custom_dve(
                        ops[(kind, var)],
                        out=scratch[:],
                        in0=base,
                        in1=bview(j, delta),
                    ).ins.perf_max = 1
                col = r * (1 + NGRP)
                nc.vector._custom_dve(
                    ops["read"],
                    out=stage[:, col : col + 1],
                    in0=scratch[:, 0:1, 0:1],
                )
            part = acc_pool.tile([128, 1], F32)
            nc.vector.tensor_reduce(
                out=part[:],
                in_=stage[:],
                axis=mybir.AxisListType.X,
                op=mybir.AluOpType.add,
            )
            nc.sync.dma_start(out=OUT[:], in_=part[:])
    return nc


_NC = None


def _get_nc():
    global _NC
    if _NC is None:
        _NC = _build_nc()
        if not _NC.is_finalized():
            _NC.finalize()
    return _NC


def _prep_shards(x: np.ndarray) -> list[dict[str, np.ndarray]]:
    """bf16-cast, circular pad, blockify into the flat per-core layout."""
    imgs = np.ascontiguousarray(x.reshape(B * C, H, W), dtype=np.float32)

    def to_bf16(a32):
        b = a32.view(np.uint32)
        return ((b + 0x7FFF + ((b >> 16) & 1)) >> 16).astype(np.uint16)

    imgs_b = to_bf16(imgs)  # (24, H, W) uint16 view of bf16
    HPAD = H + 3
    even = np.zeros((B * C, HPAD, WP), dtype=np.uint16)
    even[:, :H, BASE : BASE + W] = imgs_b
    even[:, :H, :BASE] = imgs_b[:, :, W - BASE :]
    even[:, :H, BASE + W : BASE + W + 3] = imgs_b[:, :, :3]
    even[:, H:, :] = even[:, :3, :]

    I = np.eye(128, dtype=np.float32)
    wi = to_bf16(I)
    wni = to_bf16(-I)

    shards = even.reshape(NCORES, IMGS, HPAD, WP)
    out = []
    for n in range(NCORES):
        t = shards[n].transpose(1, 0, 2)  # (HPAD, IMGS, WP)
        blk = np.empty((NBLK, ROWS_BLK, IMGS, WP), dtype=np.uint16)
        for r in range(NBLK):
            blk[r] = t[r * RB : r * RB + ROWS_BLK]
        flat = np.concatenate([blk.reshape(-1), np.zeros(8, np.uint16)])
        out.append({"x": flat, "wi": wi, "wni": wni})
    return out


def _run(x: np.ndarray, trace: bool = False):
    import ml_dtypes

    nc = _get_nc()
    in_maps = _prep_shards(x)
    in_maps = [
        {k: v.view(ml_dtypes.bfloat16) for k, v in m.items()} for m in in_maps
    ]
    res = run_bass_kernel_spmd(
        nc, in_maps, core_ids=list(range(NCORES)), trace=trace
    )
    total = 0.0
    for r in res.results:
        total += r["out"].astype(np.float64).sum()
    val = WEIGHT * 2.0 * total / float(B * C * H * W)
    return np.float32(val), res


def kernel(x: np.ndarray) -> np.ndarray:
    x = np.asarray(x, dtype=np.float32)
    val, _ = _run(x, trace=False)
    return val
